# revision 9
# baseline (speedup 1.0000x reference)
"""Causal MLA block on 8 TRN2 NeuronCores.

Sharding (token-parallel, no all-reduce):
  * kv phase: core c computes k/v projections + RoPE + per-head compression for
    the contiguous token slice [c*512, (c+1)*512) of the flattened [4096]
    tokens, then one bf16 AllGather shares the compressed kc/vc (1 MB/core)
    with every core.
  * q/attention/out phase: core c owns 4 query blocks of 128 tokens chosen so
    causal attention work is near-identical on every core:
    batch0 blocks {c+8, c}, batch1 blocks {15-c, 7-c}  (slot order).
    It computes q projection + RoPE + compression, block-causal attention in
    the 32-dim latent space, per-head decompression, and the full output
    projection for its 512 tokens. Outputs are disjoint token rows; the host
    reassembles.

SPMD needs one static program, but each core's causal extents differ, so the
kernel processes fixed per-slot kv extents [16, 8, 16, 8] chunks and the host
bakes per-core masks (ones / causal-tri / zeros per chunk) into a mask table;
fully-masked surplus chunks contribute exactly zero to both the attention
numerator and the softmax denominator (the denominator is a ones-column
augmented onto vc, so masking the exp'd scores masks the denominator too).
For the extent-16 slots, chunks 0-7 are below the diagonal for every core and
skip the mask multiply entirely.

All matmuls run in bf16 with fp32 PSUM accumulation; RoPE and softmax in fp32.
Softmax skips max-subtraction (scores are O(1), exp cannot overflow). Weights
are pre-transposed/pre-permuted on the host so the device never transposes:
q/k projection output features are permuted within each head to [even, odd]
halves so RoPE acts on contiguous partition halves; compression weights get
the matching column permutation; the 1/sqrt(32) score scale folds into Wqc.
"""

import sys

sys.path.insert(0, "/opt/trn_rl_repo")

from contextlib import ExitStack

import numpy as np
import ml_dtypes

import concourse.bass as bass
import concourse.mybir as mybir
import concourse.tile as tile
from concourse import bacc
from concourse import bass_utils

B, L, D, H = 2, 2048, 2048, 16
HD = D // H          # 128
LAT = HD // 4        # 32
P = 128
NCORES = 8
TKV = 512            # kv tokens per core
NCH = D // P         # 16 chunks of 128
FP32 = mybir.dt.float32
BF16 = mybir.dt.bfloat16
bfnp = ml_dtypes.bfloat16
AUG = LAT + 1        # 33: vc augmented with a ones column
SLOT_EXTENTS = [16, 8, 16, 8]          # compiled kv-chunk extent per q slot
SLOT_OFF = [0, 16, 24, 40]             # mask-table column offset per slot
NMASK = 48


def _qblocks(c):
    """(batch, block_idx) per q slot; block_idx+1 <= SLOT_EXTENTS[slot]."""
    return [(0, c + 8), (0, c), (1, 15 - c), (1, 7 - c)]


def build_kernel(flags):
    nc = bacc.Bacc("TRN2", target_bir_lowering=False, debug=False, num_devices=NCORES)

    ins = {}

    def din(name, shape, dt):
        ins[name] = nc.dram_tensor(name, shape, dt, kind="ExternalInput").ap()
        return ins[name]

    xkvT = din("xkvT", [D, TKV], BF16)
    xqT = din("xqT", [D, TKV], BF16)
    wqT = din("wqT", [D, D], BF16)
    wkT = din("wkT", [D, D], BF16)
    wvT = din("wvT", [D, D], BF16)
    woT = din("woT", [D, D], BF16)
    wqcT = din("wqcT", [HD, LAT], BF16)
    wkcT = din("wkcT", [HD, LAT], BF16)
    wvcT = din("wvcT", [HD, LAT], BF16)
    wdT4 = din("wdT4", [P, HD], BF16)
    ckv = din("ckv", [P, TKV], FP32)
    skv = din("skv", [P, TKV], FP32)
    cq = din("cq", [P, TKV], FP32)
    sq = din("sq", [P, TKV], FP32)
    bvc_t = din("bvc_t", [P, AUG], FP32)       # bvc broadcast + ones column
    mask_tab = din("mask_tab", [NMASK * P, P], BF16)
    for nm in ["bq", "bk", "bv", "bo"]:
        if flags[nm]:
            din(nm + "_t", [P, H], FP32)
    for nm in ["bqc", "bkc", "bd"]:
        if flags[nm]:
            din(nm + "_t", [P, 1], FP32)

    outT = nc.dram_tensor("outT", [D, TKV], FP32, kind="ExternalOutput").ap()

    with tile.TileContext(nc) as tc, ExitStack() as ctx:
        sbc = ctx.enter_context(tc.tile_pool(name="sbc", bufs=1))
        sbx = ctx.enter_context(tc.tile_pool(name="sbx", bufs=2))
        sbw = ctx.enter_context(tc.tile_pool(name="sbw", bufs=2))
        sbr = ctx.enter_context(tc.tile_pool(name="sbr", bufs=1))
        sbt = ctx.enter_context(tc.tile_pool(name="sbt", bufs=2))
        psmm = ctx.enter_context(tc.tile_pool(name="psmm", bufs=2, space="PSUM"))
        pss = ctx.enter_context(tc.tile_pool(name="pss", bufs=4, space="PSUM"))
        psav = ctx.enter_context(tc.tile_pool(name="psav", bufs=2, space="PSUM"))
        dram = ctx.enter_context(tc.tile_pool(name="dram", bufs=1, space="DRAM"))

        kv_local = dram.tile([TKV, TKV + H * AUG], BF16)   # [512, 1040]
        kv_all = dram.tile(
            [NCORES * TKV, TKV + H * AUG], BF16, addr_space="Shared"
        )

        def w_view(w):
            return w.rearrange("(dc p) o -> p dc o", p=P)

        # ---- phase A inputs first: DMA issue order follows program order ----
        xkv_sb = sbx.tile([P, NCH, TKV], BF16, name="x_sb", tag="x_sb")
        nc.sync.dma_start(xkv_sb[:], xkvT.rearrange("(dc p) t -> p dc t", p=P))
        cs = {}
        for nm, src in [("ckv", ckv), ("skv", skv)]:
            t = sbc.tile([P, TKV], FP32, name=f"cs_{nm}", tag=f"cs_{nm}")
            nc.sync.dma_start(t[:], src)
            cs[nm] = t
        wkcT_sb = sbc.tile([HD, LAT], BF16)
        nc.sync.dma_start(wkcT_sb[:], wkcT)

        biases = {}
        for nm in ["bq", "bk", "bv", "bqc", "bkc", "bd", "bo"]:
            if flags[nm]:
                shp = [P, H] if nm in ("bq", "bk", "bv", "bo") else [P, 1]
                t = sbc.tile(shp, FP32, name=f"b_{nm}", tag=f"b_{nm}")
                nc.sync.dma_start(t[:], ins[nm + "_t"])
                biases[nm] = t

        def proj_loop(w, x_sb, wname, consume):
            """for each och: psum[o(128), t(512)] = sum_dc w.T @ x; consume it.

            Weights stream in 4-och chunks ([128, 16, 512] bf16, 2 MB) to cut
            DMA-issue count on the Sync engine.
            """
            for ochg in range(NCH // 2):
                w4 = sbw.tile([P, NCH, 2 * P], BF16, name=f"w_{wname}", tag="w4")
                nc.sync.dma_start(
                    w4[:], w_view(w)[:, :, ochg * 2 * P : (ochg + 1) * 2 * P]
                )
                for ol in range(2):
                    och = ochg * 2 + ol
                    ps = psmm.tile([P, TKV], FP32, name="ps_mm", tag="ps_mm")
                    for dc in range(NCH):
                        nc.tensor.matmul(
                            ps[:],
                            lhsT=w4[:, dc, ol * P : (ol + 1) * P],
                            rhs=x_sb[:, dc, :],
                            start=(dc == 0), stop=(dc == NCH - 1),
                        )
                    consume(och, ps)

        def rope(ps, c_t, s_t, bias, och, rot_dst):
            """rot_dst[:, och, :] (bf16) = RoPE(ps); [a;b] half layout."""
            if bias is not None:
                nc.vector.tensor_scalar_add(ps[:], ps[:], bias[:, och : och + 1])
            sw = sbt.tile([P, TKV], FP32, name="sw", tag="sw")
            nc.vector.tensor_copy(sw[0:64, :], ps[64:128, :])
            nc.vector.tensor_copy(sw[64:128, :], ps[0:64, :])
            t1 = sbt.tile([P, TKV], FP32, name="t1", tag="t1")
            t2 = sbt.tile([P, TKV], FP32, name="t2", tag="t2")
            nc.vector.tensor_mul(t1[:], ps[:], c_t[:])
            nc.vector.tensor_mul(t2[:], sw[:], s_t[:])
            nc.vector.tensor_add(rot_dst[:, och, :], t1[:], t2[:])

        def compress_cT(rot_sb, wc_sb, bias_c, dst_fn):
            """[kq]cT per head-group: 4 col-packed K=128 matmuls -> [128,512]"""
            for hg in range(4):
                ps = psmm.tile([P, TKV], FP32, name="ps_c", tag="ps_mm")
                for g in range(4):
                    h = hg * 4 + g
                    nc.tensor.matmul(
                        ps[32 * g : 32 * (g + 1), :],
                        lhsT=wc_sb[:], rhs=rot_sb[:, h, :],
                        start=True, stop=True, tile_position=(0, 32 * g),
                    )
                if bias_c is not None:
                    nc.vector.tensor_scalar_add(ps[:], ps[:], bias_c[:, 0:1])
                nc.scalar.copy(dst_fn(hg), ps[:])

        # ---- phase A: k projection + RoPE + kc compression ----
        krot = sbr.tile([P, NCH, TKV], BF16, name="rot_sb", tag="rot_sb")
        proj_loop(
            wkT, xkv_sb, "k",
            lambda och, ps: rope(ps, cs["ckv"], cs["skv"], biases.get("bk"), och, krot),
        )
        kc_sb = sbc.tile([P, 4, TKV], BF16)
        compress_cT(krot, wkcT_sb, biases.get("bkc"), lambda hg: kc_sb[:, hg, :])
        nc.sync.dma_start(
            kv_local[:, 0:TKV].rearrange("(hg p) t -> p hg t", p=P), kc_sb[:]
        )

        # ---- phase A2: v projection + vc compression ----
        wvcT_sb = sbc.tile([HD, LAT], BF16)
        nc.sync.dma_start(wvcT_sb[:], wvcT)
        bvc_sb = sbc.tile([P, AUG], FP32)
        nc.sync.dma_start(bvc_sb[:], bvc_t)
        vc_sb = sbc.tile([P, TKV // P, H * AUG], BF16)  # [128, 4, 528]

        def v_consume(h, ps):
            if "bv" in biases:
                nc.vector.tensor_scalar_add(ps[:], ps[:], biases["bv"][:, h : h + 1])
            vT = sbt.tile([P, TKV], BF16, name="vT", tag="vT")
            nc.scalar.copy(vT[:], ps[:])
            for tch in range(TKV // P):
                psv = psmm.tile([P, TKV], FP32, name="ps_vc", tag="ps_mm")
                nc.tensor.matmul(
                    psv[:, 0:LAT],
                    lhsT=vT[:, tch * P : (tch + 1) * P], rhs=wvcT_sb[:],
                    start=True, stop=True,
                )
                dst = vc_sb[:, tch, h * AUG : (h + 1) * AUG]
                nc.vector.tensor_tensor(
                    dst[:, 0:LAT], psv[:, 0:LAT], bvc_sb[:, 0:LAT],
                    mybir.AluOpType.add,
                )
                nc.vector.tensor_copy(dst[:, LAT:AUG], bvc_sb[:, LAT:AUG])

        proj_loop(wvT, xkv_sb, "v", v_consume)
        nc.sync.dma_start(
            kv_local[:, TKV:].rearrange("(tch p) e -> p tch e", p=P), vc_sb[:]
        )

        # ---- AllGather of compressed kv (1 MB bf16 per core) ----
        nc.gpsimd.collective_compute(
            "AllGather", mybir.AluOpType.bypass,
            replica_groups=[list(range(NCORES))],
            ins=[kv_local.opt()], outs=[kv_all.opt()],
        )

        # ---- phase B: q projection + RoPE + qc compression (overlaps AG) ----
        xq_sb = sbx.tile([P, NCH, TKV], BF16, name="x_sb", tag="x_sb")
        nc.sync.dma_start(xq_sb[:], xqT.rearrange("(dc p) t -> p dc t", p=P))
        for nm, src in [("cq", cq), ("sq", sq)]:
            t = sbc.tile([P, TKV], FP32, name=f"cs_{nm}", tag=f"cs_{nm}")
            nc.sync.dma_start(t[:], src)
            cs[nm] = t
        wqcT_sb = sbc.tile([HD, LAT], BF16)
        nc.sync.dma_start(wqcT_sb[:], wqcT)
        qrot = sbr.tile([P, NCH, TKV], BF16, name="rot_sb", tag="rot_sb")
        proj_loop(
            wqT, xq_sb, "q",
            lambda och, ps: rope(ps, cs["cq"], cs["sq"], biases.get("bq"), och, qrot),
        )
        qcT_sb = [
            sbc.tile([P, TKV], BF16, name=f"qct{hg}", tag=f"qct{hg}") for hg in range(4)
        ]
        compress_cT(qrot, wqcT_sb, biases.get("bqc"), lambda hg: qcT_sb[hg][:])

        # ---- phase C inputs ----
        mask_sb = sbc.tile([P, NMASK, P], BF16)
        nc.sync.dma_start(mask_sb[:], mask_tab.rearrange("(s p) n -> p s n", p=P))
        wdT4_sb = sbc.tile([P, HD], BF16)
        nc.sync.dma_start(wdT4_sb[:], wdT4)

        # decompressed-attention buffers (transposed), one per head
        attnDT = [
            sbc.tile([P, TKV], BF16, name=f"adt{h}", tag=f"adt{h}") for h in range(H)
        ]

        kc_view = kv_all[:, 0:TKV].rearrange("(r blk p) t -> p r blk t", blk=4, p=P)

        def outproj_half(bb):
            """out columns for batch bb's two q slots (256 wide)."""
            c0 = 2 * bb * P
            for ochg in range(NCH // 2):
                w4 = sbw.tile([P, NCH, 2 * P], BF16, name="w_o", tag="w4")
                nc.sync.dma_start(
                    w4[:], w_view(woT)[:, :, ochg * 2 * P : (ochg + 1) * 2 * P]
                )
                for ol in range(2):
                    och = ochg * 2 + ol
                    ps = psmm.tile([P, TKV], FP32, name="ps_o", tag="ps_mm")
                    for h in range(H):
                        nc.tensor.matmul(
                            ps[:, 0 : 2 * P],
                            lhsT=w4[:, h, ol * P : (ol + 1) * P],
                            rhs=attnDT[h][:, c0 : c0 + 2 * P],
                            start=(h == 0), stop=(h == H - 1),
                        )
                    oe = sbt.tile([P, 2 * P], FP32, name="oe", tag="oe")
                    if "bo" in biases:
                        nc.vector.tensor_scalar_add(
                            oe[:], ps[:, 0 : 2 * P], biases["bo"][:, och : och + 1]
                        )
                    else:
                        nc.scalar.copy(oe[:], ps[:, 0 : 2 * P])
                    nc.sync.dma_start(
                        outT[och * P : (och + 1) * P, c0 : c0 + 2 * P], oe[:]
                    )

        # ---- phase C: attention (+ interleaved output projection halves) ----
        for b in range(B):
            for hg in range(4):
                kct = sbt.tile([P, 4, TKV], BF16, name="kct", tag="kct")
                nc.sync.dma_start(kct[:], kc_view[:, 4 * b : 4 * b + 4, hg, :])
                vct = sbt.tile([P, L // P, 4 * AUG], BF16, name="vct", tag="vct")
                nc.sync.dma_start(
                    vct[:],
                    kv_all[
                        b * L : (b + 1) * L,
                        TKV + hg * 4 * AUG : TKV + (hg + 1) * 4 * AUG,
                    ].rearrange("(ch p) e -> p ch e", p=P),
                )
                for qi in range(2):
                    qslot = 2 * b + qi
                    n_kv = SLOT_EXTENTS[qslot]
                    n_grp = (n_kv + 3) // 4
                    expt = [
                        sbt.tile(
                            [P, n_kv, P], BF16, name=f"expt{g}", tag=f"expt{g}",
                            bufs=2,
                        )
                        for g in range(4)
                    ]
                    for grp in range(n_grp):
                        nch = min(4, n_kv - grp * 4)
                        pst = [
                            pss.tile([P, TKV], FP32, name=f"ps_s{g}", tag="ps_s")
                            for g in range(4)
                        ]
                        for ci in range(nch):
                            ch = grp * 4 + ci
                            for g in range(4):
                                nc.tensor.matmul(
                                    pst[g][:, ci * P : (ci + 1) * P],
                                    lhsT=kct[
                                        32 * g : 32 * (g + 1),
                                        ch // 4,
                                        (ch % 4) * P : (ch % 4 + 1) * P,
                                    ],
                                    rhs=qcT_sb[hg][
                                        32 * g : 32 * (g + 1),
                                        qslot * P : (qslot + 1) * P,
                                    ],
                                    start=True, stop=True,
                                    tile_position=(32 * g, 0),
                                )
                        # exp (fp32 PSUM -> bf16 SBUF); mask only where some
                        # core needs it: extent-16 slots have chunks 0-7
                        # strictly below every core's diagonal (all-ones mask)
                        need_mask = not (n_kv == 16 and grp < 2)
                        m0 = SLOT_OFF[qslot] + grp * 4
                        msl = mask_sb[:, m0 : m0 + nch, :]
                        for g in range(4):
                            nc.scalar.activation(
                                expt[g][:, grp * 4 : grp * 4 + nch, :],
                                pst[g][:, 0 : nch * P],
                                mybir.ActivationFunctionType.Exp,
                            )
                            if need_mask:
                                nc.vector.tensor_mul(
                                    expt[g][:, grp * 4 : grp * 4 + nch, :],
                                    expt[g][:, grp * 4 : grp * 4 + nch, :],
                                    msl,
                                )
                    # av: accumulate attnT_aug over chunks; 2 heads per bank
                    pav = [
                        psav.tile([P, P], FP32, name=f"ps_av{pr}", tag="ps_av")
                        for pr in range(2)
                    ]
                    for ch in range(n_kv):
                        for g in range(4):
                            pr, s = g // 2, g % 2
                            nc.tensor.matmul(
                                pav[pr][64 * s : 64 * s + AUG, :],
                                lhsT=vct[:, ch, g * AUG : (g + 1) * AUG],
                                rhs=expt[g][:, ch, :],
                                start=(ch == 0), stop=(ch == n_kv - 1),
                                tile_position=(0, 64 * s),
                            )
                    # normalize by the ones-column sum, then decompress
                    anorm = sbt.tile([P, P], BF16, name="anorm", tag="anorm")
                    for g in range(4):
                        pr, s = g // 2, g % 2
                        den = sbt.tile([1, P], FP32, name="den", tag="den", bufs=4)
                        nc.vector.tensor_copy(
                            den[:], pav[pr][64 * s + LAT : 64 * s + LAT + 1, :]
                        )
                        rc = sbt.tile([1, P], FP32, name="rc", tag="rc", bufs=4)
                        nc.vector.reciprocal_approx_fast(rc[:], den[:])
                        rcb = sbt.tile([LAT, P], FP32, name="rcb", tag="rcb", bufs=4)
                        nc.gpsimd.partition_broadcast(rcb[:], rc[:])
                        nc.vector.tensor_mul(
                            anorm[32 * g : 32 * (g + 1), :],
                            pav[pr][64 * s : 64 * s + LAT, :],
                            rcb[:],
                        )
                    for g in range(4):
                        h = hg * 4 + g
                        psd = pss.tile([P, TKV], FP32, name="ps_d", tag="ps_s")
                        nc.tensor.matmul(
                            psd[:, 0:P],
                            lhsT=wdT4_sb[32 * g : 32 * (g + 1), :],
                            rhs=anorm[32 * g : 32 * (g + 1), :],
                            start=True, stop=True, tile_position=(32 * g, 0),
                        )
                        dst = attnDT[h][:, qslot * P : (qslot + 1) * P]
                        if "bd" in biases:
                            nc.vector.tensor_scalar_add(
                                dst, psd[:, 0:P], biases["bd"][:, 0:1]
                            )
                        else:
                            nc.scalar.copy(dst, psd[:, 0:P])
            # output projection for this batch's 256 columns overlaps the
            # next batch's attention
            outproj_half(b)

    nc.compile()
    return nc


_NC_CACHE = {}


def _get_nc(flags):
    key = tuple(sorted(flags.items()))
    if key not in _NC_CACHE:
        _NC_CACHE[key] = build_kernel(flags)
    return _NC_CACHE[key]


def _prep_inputs(inputs):
    f32 = np.float32
    x = np.asarray(inputs["x"], f32).reshape(B * L, D)
    fc = np.asarray(inputs["freqs_cis"], f32)
    cr, ci = fc[:, :, 0], fc[:, :, 1]

    perm = np.concatenate([np.arange(0, HD, 2), np.arange(1, HD, 2)])
    permD = (np.arange(D) // HD) * HD + perm[np.arange(D) % HD]

    Wq = np.asarray(inputs["Wq"], f32)[permD]
    Wk = np.asarray(inputs["Wk"], f32)[permD]
    Wv = np.asarray(inputs["Wv"], f32)
    Wo = np.asarray(inputs["Wo"], f32)
    scale = f32(1.0 / np.sqrt(LAT))
    Wqc = np.asarray(inputs["Wqc"], f32)[:, perm] * scale
    Wkc = np.asarray(inputs["Wkc"], f32)[:, perm]
    Wvc = np.asarray(inputs["Wvc"], f32)
    Wd = np.asarray(inputs["Wd"], f32)
    bq = np.asarray(inputs["bq"], f32)[permD]
    bk = np.asarray(inputs["bk"], f32)[permD]
    bv = np.asarray(inputs["bv"], f32)
    bqc = np.asarray(inputs["bqc"], f32) * scale
    bkc = np.asarray(inputs["bkc"], f32)
    bvc = np.asarray(inputs["bvc"], f32)
    bd = np.asarray(inputs["bd"], f32)
    bo = np.asarray(inputs["bo"], f32)

    flags = dict(
        bq=bool(np.any(bq)), bk=bool(np.any(bk)), bv=bool(np.any(bv)),
        bqc=bool(np.any(bqc)), bkc=bool(np.any(bkc)), bd=bool(np.any(bd)),
        bo=bool(np.any(bo)),
    )

    shared = dict(
        wqT=np.ascontiguousarray(Wq.T).astype(bfnp),
        wkT=np.ascontiguousarray(Wk.T).astype(bfnp),
        wvT=np.ascontiguousarray(Wv.T).astype(bfnp),
        woT=np.ascontiguousarray(Wo.T).astype(bfnp),
        wqcT=np.ascontiguousarray(Wqc.T).astype(bfnp),
        wkcT=np.ascontiguousarray(Wkc.T).astype(bfnp),
        wvcT=np.ascontiguousarray(Wvc.T).astype(bfnp),
        wdT4=np.ascontiguousarray(np.tile(Wd.T, (4, 1))).astype(bfnp),
        bvc_t=np.concatenate(
            [np.tile(bvc, (P, 1)), np.ones((P, 1), f32)], axis=1
        ).astype(f32),
    )
    if flags["bq"]:
        shared["bq_t"] = np.ascontiguousarray(bq.reshape(H, HD).T).astype(f32)
    if flags["bk"]:
        shared["bk_t"] = np.ascontiguousarray(bk.reshape(H, HD).T).astype(f32)
    if flags["bv"]:
        shared["bv_t"] = np.ascontiguousarray(bv.reshape(H, HD).T).astype(f32)
    if flags["bqc"]:
        shared["bqc_t"] = np.tile(bqc, 4)[:, None].astype(f32)
    if flags["bkc"]:
        shared["bkc_t"] = np.tile(bkc, 4)[:, None].astype(f32)
    if flags["bd"]:
        shared["bd_t"] = bd[:, None].astype(f32)
    if flags["bo"]:
        shared["bo_t"] = np.ascontiguousarray(bo.reshape(H, HD).T).astype(f32)

    tri = (np.arange(P)[None, :] >= np.arange(P)[:, None]).astype(f32)
    ones = np.ones((P, P), f32)
    zeros = np.zeros((P, P), f32)

    in_maps = []
    for c in range(NCORES):
        m = dict(shared)
        tk0 = c * TKV
        pos_kv = np.arange(TKV) + (tk0 % L)
        m["xkvT"] = np.ascontiguousarray(x[tk0 : tk0 + TKV].T).astype(bfnp)
        m["ckv"] = np.vstack([cr[pos_kv].T, cr[pos_kv].T]).astype(f32)
        m["skv"] = np.vstack([-ci[pos_kv].T, ci[pos_kv].T]).astype(f32)

        qb = _qblocks(c)
        toks = np.concatenate(
            [np.arange(blk * P, (blk + 1) * P) + bb * L for (bb, blk) in qb]
        )
        pos_q = toks % L
        m["xqT"] = np.ascontiguousarray(x[toks].T).astype(bfnp)
        m["cq"] = np.vstack([cr[pos_q].T, cr[pos_q].T]).astype(f32)
        m["sq"] = np.vstack([-ci[pos_q].T, ci[pos_q].T]).astype(f32)

        mtab = np.zeros((NMASK * P, P), f32)
        for qi, (bb, blk) in enumerate(qb):
            for ch in range(SLOT_EXTENTS[qi]):
                blkv = ones if ch < blk else (tri if ch == blk else zeros)
                row0 = (SLOT_OFF[qi] + ch) * P
                mtab[row0 : row0 + P] = blkv
        m["mask_tab"] = mtab.astype(bfnp)
        in_maps.append(m)

    return flags, in_maps


def _assemble(results):
    out = np.empty((B, L, D), np.float32)
    for c in range(NCORES):
        oT = results[c]["outT"]
        for qi, (bb, blk) in enumerate(_qblocks(c)):
            out[bb, blk * P : (blk + 1) * P, :] = oT[:, qi * P : (qi + 1) * P].T
    return out


def run(inputs, trace=False):
    flags, in_maps = _prep_inputs(inputs)
    nc = _get_nc(flags)
    res = bass_utils.run_bass_kernel_spmd(
        nc, in_maps, core_ids=list(range(NCORES)), trace=trace
    )
    return _assemble(res.results), res


def kernel(**inputs):
    out, _ = run(inputs)
    return out


# revision 11
# speedup vs baseline: 1.0260x; 1.0260x over previous
"""Causal MLA block on 8 TRN2 NeuronCores.

Sharding (token-parallel, no all-reduce):
  * kv phase: core c computes k/v projections + RoPE + per-head compression for
    the contiguous token slice [c*512, (c+1)*512) of the flattened [4096]
    tokens, then one bf16 AllGather shares the compressed kc/vc (1 MB/core)
    with every core.
  * q/attention/out phase: core c owns 4 query blocks of 128 tokens chosen so
    causal attention work is near-identical on every core:
    batch0 blocks {c+8, c}, batch1 blocks {15-c, 7-c}  (slot order).
    It computes q projection + RoPE + compression, block-causal attention in
    the 32-dim latent space, per-head decompression, and the full output
    projection for its 512 tokens. Outputs are disjoint token rows; the host
    reassembles.

SPMD needs one static program, but each core's causal extents differ, so the
kernel processes fixed per-slot kv extents [16, 8, 16, 8] chunks and the host
bakes per-core masks (ones / causal-tri / zeros per chunk) into a mask table;
fully-masked surplus chunks contribute exactly zero to both the attention
numerator and the softmax denominator (the denominator is a ones-column
augmented onto vc, so masking the exp'd scores masks the denominator too).
For the extent-16 slots, chunks 0-7 are below the diagonal for every core and
skip the mask multiply entirely.

All matmuls run in bf16 with fp32 PSUM accumulation; RoPE and softmax in fp32.
Softmax skips max-subtraction (scores are O(1), exp cannot overflow). Weights
are pre-transposed/pre-permuted on the host so the device never transposes:
q/k projection output features are permuted within each head to [even, odd]
halves so RoPE acts on contiguous partition halves; compression weights get
the matching column permutation; the 1/sqrt(32) score scale folds into Wqc.
"""

import sys

sys.path.insert(0, "/opt/trn_rl_repo")

from contextlib import ExitStack

import numpy as np
import ml_dtypes

import concourse.bass as bass
import concourse.mybir as mybir
import concourse.tile as tile
from concourse import bacc
from concourse import bass_utils

B, L, D, H = 2, 2048, 2048, 16
HD = D // H          # 128
LAT = HD // 4        # 32
P = 128
NCORES = 8
TKV = 512            # kv tokens per core
NCH = D // P         # 16 chunks of 128
FP32 = mybir.dt.float32
BF16 = mybir.dt.bfloat16
bfnp = ml_dtypes.bfloat16
AUG = LAT + 1        # 33: vc augmented with a ones column
SLOT_EXTENTS = [16, 8, 16, 8]          # compiled kv-chunk extent per q slot
SLOT_OFF = [0, 16, 24, 40]             # mask-table column offset per slot
NMASK = 48


def _qblocks(c):
    """(batch, block_idx) per q slot; block_idx+1 <= SLOT_EXTENTS[slot]."""
    return [(0, c + 8), (0, c), (1, 15 - c), (1, 7 - c)]


def build_kernel(flags):
    nc = bacc.Bacc("TRN2", target_bir_lowering=False, debug=False, num_devices=NCORES)

    ins = {}

    def din(name, shape, dt):
        ins[name] = nc.dram_tensor(name, shape, dt, kind="ExternalInput").ap()
        return ins[name]

    xkvT = din("xkvT", [D, TKV], BF16)
    xqT = din("xqT", [D, TKV], BF16)
    wqT = din("wqT", [D, D], BF16)
    wkT = din("wkT", [D, D], BF16)
    wvT = din("wvT", [D, D], BF16)
    woT = din("woT", [D, D], BF16)
    wqcT = din("wqcT", [HD, LAT], BF16)
    wkcT = din("wkcT", [HD, LAT], BF16)
    wvcT = din("wvcT", [HD, LAT], BF16)
    wdT4 = din("wdT4", [P, HD], BF16)
    ckv = din("ckv", [P, TKV], FP32)
    skv = din("skv", [P, TKV], FP32)
    cq = din("cq", [P, TKV], FP32)
    sq = din("sq", [P, TKV], FP32)
    bvc_t = din("bvc_t", [P, AUG], FP32)       # bvc broadcast + ones column
    mask_tab = din("mask_tab", [NMASK * P, P], BF16)
    for nm in ["bq", "bk", "bv", "bo"]:
        if flags[nm]:
            din(nm + "_t", [P, H], FP32)
    for nm in ["bqc", "bkc", "bd"]:
        if flags[nm]:
            din(nm + "_t", [P, 1], FP32)

    outT = nc.dram_tensor("outT", [D, TKV], FP32, kind="ExternalOutput").ap()

    with tile.TileContext(nc) as tc, ExitStack() as ctx:
        sbc = ctx.enter_context(tc.tile_pool(name="sbc", bufs=1))
        sbx = ctx.enter_context(tc.tile_pool(name="sbx", bufs=2))
        sbw = ctx.enter_context(tc.tile_pool(name="sbw", bufs=2))
        sbr = ctx.enter_context(tc.tile_pool(name="sbr", bufs=1))
        sbt = ctx.enter_context(tc.tile_pool(name="sbt", bufs=2))
        psmm = ctx.enter_context(tc.tile_pool(name="psmm", bufs=2, space="PSUM"))
        pss = ctx.enter_context(tc.tile_pool(name="pss", bufs=4, space="PSUM"))
        psav = ctx.enter_context(tc.tile_pool(name="psav", bufs=2, space="PSUM"))
        dram = ctx.enter_context(tc.tile_pool(name="dram", bufs=1, space="DRAM"))

        kv_local = dram.tile([TKV, TKV + H * AUG], BF16)   # [512, 1040]
        kv_all = dram.tile(
            [NCORES * TKV, TKV + H * AUG], BF16, addr_space="Shared"
        )

        def w_view(w):
            return w.rearrange("(dc p) o -> p dc o", p=P)

        # ---- phase A inputs first: DMA issue order follows program order ----
        xkv_sb = sbx.tile([P, NCH, TKV], BF16, name="x_sb", tag="x_sb")
        nc.sync.dma_start(xkv_sb[:], xkvT.rearrange("(dc p) t -> p dc t", p=P))
        cs = {}
        for nm, src in [("ckv", ckv), ("skv", skv)]:
            t = sbc.tile([P, TKV], FP32, name=f"cs_{nm}", tag=f"cs_{nm}")
            nc.sync.dma_start(t[:], src)
            cs[nm] = t
        wkcT_sb = sbc.tile([HD, LAT], BF16)
        nc.sync.dma_start(wkcT_sb[:], wkcT)

        biases = {}
        for nm in ["bq", "bk", "bv", "bqc", "bkc", "bd", "bo"]:
            if flags[nm]:
                shp = [P, H] if nm in ("bq", "bk", "bv", "bo") else [P, 1]
                t = sbc.tile(shp, FP32, name=f"b_{nm}", tag=f"b_{nm}")
                nc.sync.dma_start(t[:], ins[nm + "_t"])
                biases[nm] = t

        def proj_loop(w, x_sb, wname, consume):
            """for each och: psum[o(128), t(512)] = sum_dc w.T @ x; consume it.

            Weights stream in 4-och chunks ([128, 16, 512] bf16, 2 MB) to cut
            DMA-issue count on the Sync engine.
            """
            for ochg in range(NCH // 2):
                w4 = sbw.tile([P, NCH, 2 * P], BF16, name=f"w_{wname}", tag="w4")
                nc.sync.dma_start(
                    w4[:], w_view(w)[:, :, ochg * 2 * P : (ochg + 1) * 2 * P]
                )
                for ol in range(2):
                    och = ochg * 2 + ol
                    ps = psmm.tile([P, TKV], FP32, name="ps_mm", tag="ps_mm")
                    for dc in range(NCH):
                        nc.tensor.matmul(
                            ps[:],
                            lhsT=w4[:, dc, ol * P : (ol + 1) * P],
                            rhs=x_sb[:, dc, :],
                            start=(dc == 0), stop=(dc == NCH - 1),
                        )
                    consume(och, ps)

        def rope(ps, c_t, s_t, bias, och, rot_dst):
            """rot_dst[:, och, :] (bf16) = RoPE(ps); [a;b] half layout."""
            if bias is not None:
                nc.vector.tensor_scalar_add(ps[:], ps[:], bias[:, och : och + 1])
            sw = sbt.tile([P, TKV], FP32, name="sw", tag="sw")
            nc.vector.tensor_copy(sw[0:64, :], ps[64:128, :])
            nc.vector.tensor_copy(sw[64:128, :], ps[0:64, :])
            t1 = sbt.tile([P, TKV], FP32, name="t1", tag="t1")
            t2 = sbt.tile([P, TKV], FP32, name="t2", tag="t2")
            nc.vector.tensor_mul(t1[:], ps[:], c_t[:])
            nc.vector.tensor_mul(t2[:], sw[:], s_t[:])
            nc.vector.tensor_add(rot_dst[:, och, :], t1[:], t2[:])

        def compress_cT(rot_sb, wc_sb, bias_c, dst_fn):
            """[kq]cT per head-group: 4 col-packed K=128 matmuls -> [128,512]"""
            for hg in range(4):
                ps = psmm.tile([P, TKV], FP32, name="ps_c", tag="ps_mm")
                for g in range(4):
                    h = hg * 4 + g
                    nc.tensor.matmul(
                        ps[32 * g : 32 * (g + 1), :],
                        lhsT=wc_sb[:], rhs=rot_sb[:, h, :],
                        start=True, stop=True, tile_position=(0, 32 * g),
                    )
                if bias_c is not None:
                    nc.vector.tensor_scalar_add(ps[:], ps[:], bias_c[:, 0:1])
                nc.scalar.copy(dst_fn(hg), ps[:])

        # ---- phase A: k projection + RoPE + kc compression ----
        krot = sbr.tile([P, NCH, TKV], BF16, name="rot_sb", tag="rot_sb")
        proj_loop(
            wkT, xkv_sb, "k",
            lambda och, ps: rope(ps, cs["ckv"], cs["skv"], biases.get("bk"), och, krot),
        )
        kc_sb = sbc.tile([P, 4, TKV], BF16)
        compress_cT(krot, wkcT_sb, biases.get("bkc"), lambda hg: kc_sb[:, hg, :])
        nc.gpsimd.dma_start(
            kv_local[:, 0:TKV].rearrange("(hg p) t -> p hg t", p=P), kc_sb[:]
        )

        # ---- phase A2: v projection + vc compression ----
        wvcT_sb = sbc.tile([HD, LAT], BF16)
        nc.sync.dma_start(wvcT_sb[:], wvcT)
        bvc_sb = sbc.tile([P, AUG], FP32)
        nc.sync.dma_start(bvc_sb[:], bvc_t)
        vc_sb = sbc.tile([P, TKV // P, H * AUG], BF16)  # [128, 4, 528]

        def v_consume(h, ps):
            if "bv" in biases:
                nc.vector.tensor_scalar_add(ps[:], ps[:], biases["bv"][:, h : h + 1])
            vT = sbt.tile([P, TKV], BF16, name="vT", tag="vT")
            nc.scalar.copy(vT[:], ps[:])
            for tch in range(TKV // P):
                psv = psmm.tile([P, TKV], FP32, name="ps_vc", tag="ps_mm")
                nc.tensor.matmul(
                    psv[:, 0:LAT],
                    lhsT=vT[:, tch * P : (tch + 1) * P], rhs=wvcT_sb[:],
                    start=True, stop=True,
                )
                dst = vc_sb[:, tch, h * AUG : (h + 1) * AUG]
                nc.vector.tensor_tensor(
                    dst[:, 0:LAT], psv[:, 0:LAT], bvc_sb[:, 0:LAT],
                    mybir.AluOpType.add,
                )
                nc.vector.tensor_copy(dst[:, LAT:AUG], bvc_sb[:, LAT:AUG])

        proj_loop(wvT, xkv_sb, "v", v_consume)
        nc.gpsimd.dma_start(
            kv_local[:, TKV:].rearrange("(tch p) e -> p tch e", p=P), vc_sb[:]
        )

        # ---- AllGather of compressed kv (1 MB bf16 per core) ----
        nc.gpsimd.collective_compute(
            "AllGather", mybir.AluOpType.bypass,
            replica_groups=[list(range(NCORES))],
            ins=[kv_local.opt()], outs=[kv_all.opt()],
        )

        # ---- phase B/C interleaved: per head-group, q projection (PE-dense)
        # then attention (ACT-dense) so the TensorEngine never idles long
        # enough for the HAM clock gate to re-throttle.
        xq_sb = sbx.tile([P, NCH, TKV], BF16, name="x_sb", tag="x_sb")
        nc.sync.dma_start(xq_sb[:], xqT.rearrange("(dc p) t -> p dc t", p=P))
        for nm, src in [("cq", cq), ("sq", sq)]:
            t = sbc.tile([P, TKV], FP32, name=f"cs_{nm}", tag=f"cs_{nm}")
            nc.sync.dma_start(t[:], src)
            cs[nm] = t
        wqcT_sb = sbc.tile([HD, LAT], BF16)
        nc.sync.dma_start(wqcT_sb[:], wqcT)
        mask_sb = sbc.tile([P, NMASK, P], BF16)
        nc.sync.dma_start(mask_sb[:], mask_tab.rearrange("(s p) n -> p s n", p=P))
        wdT4_sb = sbc.tile([P, HD], BF16)
        nc.sync.dma_start(wdT4_sb[:], wdT4)

        qrot = sbr.tile([P, NCH, TKV], BF16, name="rot_sb", tag="rot_sb")
        qcT_sb = [
            sbc.tile([P, TKV], BF16, name=f"qct{hg}", tag=f"qct{hg}") for hg in range(4)
        ]
        attnDT = [
            sbc.tile([P, TKV], BF16, name=f"adt{h}", tag=f"adt{h}") for h in range(H)
        ]
        kc_view = kv_all[:, 0:TKV].rearrange("(r blk p) t -> p r blk t", blk=4, p=P)

        def attention_bhg(b, hg):
            kct = sbt.tile([P, 4, TKV], BF16, name="kct", tag="kct")
            nc.gpsimd.dma_start(kct[:], kc_view[:, 4 * b : 4 * b + 4, hg, :])
            vct = sbt.tile([P, L // P, 4 * AUG], BF16, name="vct", tag="vct")
            nc.gpsimd.dma_start(
                vct[:],
                kv_all[
                    b * L : (b + 1) * L,
                    TKV + hg * 4 * AUG : TKV + (hg + 1) * 4 * AUG,
                ].rearrange("(ch p) e -> p ch e", p=P),
            )
            for qi in range(2):
                qslot = 2 * b + qi
                n_kv = SLOT_EXTENTS[qslot]
                n_grp = (n_kv + 3) // 4
                expt = [
                    sbt.tile(
                        [P, n_kv, P], BF16, name=f"expt{g}", tag=f"expt{g}",
                        bufs=2,
                    )
                    for g in range(4)
                ]
                for grp in range(n_grp):
                    nch = min(4, n_kv - grp * 4)
                    pst = [
                        pss.tile([P, TKV], FP32, name=f"ps_s{g}", tag="ps_s")
                        for g in range(4)
                    ]
                    for ci in range(nch):
                        ch = grp * 4 + ci
                        for g in range(4):
                            nc.tensor.matmul(
                                pst[g][:, ci * P : (ci + 1) * P],
                                lhsT=kct[
                                    32 * g : 32 * (g + 1),
                                    ch // 4,
                                    (ch % 4) * P : (ch % 4 + 1) * P,
                                ],
                                rhs=qcT_sb[hg][
                                    32 * g : 32 * (g + 1),
                                    qslot * P : (qslot + 1) * P,
                                ],
                                start=True, stop=True,
                                tile_position=(32 * g, 0),
                            )
                    # exp (fp32 PSUM -> bf16 SBUF); mask only where some core
                    # needs it: extent-16 slots have chunks 0-7 strictly below
                    # every core's diagonal (all-ones mask)
                    need_mask = not (n_kv == 16 and grp < 2)
                    m0 = SLOT_OFF[qslot] + grp * 4
                    msl = mask_sb[:, m0 : m0 + nch, :]
                    for g in range(4):
                        nc.scalar.activation(
                            expt[g][:, grp * 4 : grp * 4 + nch, :],
                            pst[g][:, 0 : nch * P],
                            mybir.ActivationFunctionType.Exp,
                        )
                        if need_mask:
                            nc.vector.tensor_mul(
                                expt[g][:, grp * 4 : grp * 4 + nch, :],
                                expt[g][:, grp * 4 : grp * 4 + nch, :],
                                msl,
                            )
                # av: accumulate attnT_aug over chunks; 2 heads per bank
                pav = [
                    psav.tile([P, P], FP32, name=f"ps_av{pr}", tag="ps_av")
                    for pr in range(2)
                ]
                for ch in range(n_kv):
                    for g in range(4):
                        pr, sgn = g // 2, g % 2
                        nc.tensor.matmul(
                            pav[pr][64 * sgn : 64 * sgn + AUG, :],
                            lhsT=vct[:, ch, g * AUG : (g + 1) * AUG],
                            rhs=expt[g][:, ch, :],
                            start=(ch == 0), stop=(ch == n_kv - 1),
                            tile_position=(0, 64 * sgn),
                        )
                # normalize by the ones-column sum, then decompress
                anorm = sbt.tile([P, P], BF16, name="anorm", tag="anorm")
                for g in range(4):
                    pr, sgn = g // 2, g % 2
                    den = sbt.tile([1, P], FP32, name="den", tag="den", bufs=4)
                    nc.vector.tensor_copy(
                        den[:], pav[pr][64 * sgn + LAT : 64 * sgn + LAT + 1, :]
                    )
                    rc = sbt.tile([1, P], FP32, name="rc", tag="rc", bufs=4)
                    nc.vector.reciprocal_approx_fast(rc[:], den[:])
                    rcb = sbt.tile([LAT, P], FP32, name="rcb", tag="rcb", bufs=4)
                    nc.gpsimd.partition_broadcast(rcb[:], rc[:])
                    nc.vector.tensor_mul(
                        anorm[32 * g : 32 * (g + 1), :],
                        pav[pr][64 * sgn : 64 * sgn + LAT, :],
                        rcb[:],
                    )
                for g in range(4):
                    h = hg * 4 + g
                    psd = pss.tile([P, TKV], FP32, name="ps_d", tag="ps_s")
                    nc.tensor.matmul(
                        psd[:, 0:P],
                        lhsT=wdT4_sb[32 * g : 32 * (g + 1), :],
                        rhs=anorm[32 * g : 32 * (g + 1), :],
                        start=True, stop=True, tile_position=(32 * g, 0),
                    )
                    dst = attnDT[h][:, qslot * P : (qslot + 1) * P]
                    if "bd" in biases:
                        nc.vector.tensor_scalar_add(
                            dst, psd[:, 0:P], biases["bd"][:, 0:1]
                        )
                    else:
                        nc.scalar.copy(dst, psd[:, 0:P])

        def outproj_half(bb):
            """out columns for batch bb's two q slots (256 wide)."""
            c0 = 2 * bb * P
            for ochg in range(NCH // 2):
                w4 = sbw.tile([P, NCH, 2 * P], BF16, name="w_o", tag="w4")
                nc.sync.dma_start(
                    w4[:], w_view(woT)[:, :, ochg * 2 * P : (ochg + 1) * 2 * P]
                )
                for ol in range(2):
                    och = ochg * 2 + ol
                    ps = psmm.tile([P, TKV], FP32, name="ps_o", tag="ps_mm")
                    for h in range(H):
                        nc.tensor.matmul(
                            ps[:, 0 : 2 * P],
                            lhsT=w4[:, h, ol * P : (ol + 1) * P],
                            rhs=attnDT[h][:, c0 : c0 + 2 * P],
                            start=(h == 0), stop=(h == H - 1),
                        )
                    oe = sbt.tile([P, 2 * P], FP32, name="oe", tag="oe")
                    if "bo" in biases:
                        nc.vector.tensor_scalar_add(
                            oe[:], ps[:, 0 : 2 * P], biases["bo"][:, och : och + 1]
                        )
                    else:
                        nc.scalar.copy(oe[:], ps[:, 0 : 2 * P])
                    nc.gpsimd.dma_start(
                        outT[och * P : (och + 1) * P, c0 : c0 + 2 * P], oe[:]
                    )

        for hg in range(4):
            # q projection for this head-group's 4 output chunks
            for ochg in (2 * hg, 2 * hg + 1):
                w4 = sbw.tile([P, NCH, 2 * P], BF16, name="w_q", tag="w4")
                nc.sync.dma_start(
                    w4[:], w_view(wqT)[:, :, ochg * 2 * P : (ochg + 1) * 2 * P]
                )
                for ol in range(2):
                    och = ochg * 2 + ol
                    ps = psmm.tile([P, TKV], FP32, name="ps_mm", tag="ps_mm")
                    for dc in range(NCH):
                        nc.tensor.matmul(
                            ps[:],
                            lhsT=w4[:, dc, ol * P : (ol + 1) * P],
                            rhs=xq_sb[:, dc, :],
                            start=(dc == 0), stop=(dc == NCH - 1),
                        )
                    rope(ps, cs["cq"], cs["sq"], biases.get("bq"), och, qrot)
            # qc compression for this head-group only
            psq = psmm.tile([P, TKV], FP32, name="ps_c", tag="ps_mm")
            for g in range(4):
                h = hg * 4 + g
                nc.tensor.matmul(
                    psq[32 * g : 32 * (g + 1), :],
                    lhsT=wqcT_sb[:], rhs=qrot[:, h, :],
                    start=True, stop=True, tile_position=(0, 32 * g),
                )
            if "bqc" in biases:
                nc.vector.tensor_scalar_add(psq[:], psq[:], biases["bqc"][:, 0:1])
            nc.scalar.copy(qcT_sb[hg][:], psq[:])
            # attention for both batches of this head-group
            for b in range(B):
                attention_bhg(b, hg)

        outproj_half(0)
        outproj_half(1)

    nc.compile()
    return nc


_NC_CACHE = {}


def _get_nc(flags):
    key = tuple(sorted(flags.items()))
    if key not in _NC_CACHE:
        _NC_CACHE[key] = build_kernel(flags)
    return _NC_CACHE[key]


def _prep_inputs(inputs):
    f32 = np.float32
    x = np.asarray(inputs["x"], f32).reshape(B * L, D)
    fc = np.asarray(inputs["freqs_cis"], f32)
    cr, ci = fc[:, :, 0], fc[:, :, 1]

    perm = np.concatenate([np.arange(0, HD, 2), np.arange(1, HD, 2)])
    permD = (np.arange(D) // HD) * HD + perm[np.arange(D) % HD]

    Wq = np.asarray(inputs["Wq"], f32)[permD]
    Wk = np.asarray(inputs["Wk"], f32)[permD]
    Wv = np.asarray(inputs["Wv"], f32)
    Wo = np.asarray(inputs["Wo"], f32)
    scale = f32(1.0 / np.sqrt(LAT))
    Wqc = np.asarray(inputs["Wqc"], f32)[:, perm] * scale
    Wkc = np.asarray(inputs["Wkc"], f32)[:, perm]
    Wvc = np.asarray(inputs["Wvc"], f32)
    Wd = np.asarray(inputs["Wd"], f32)
    bq = np.asarray(inputs["bq"], f32)[permD]
    bk = np.asarray(inputs["bk"], f32)[permD]
    bv = np.asarray(inputs["bv"], f32)
    bqc = np.asarray(inputs["bqc"], f32) * scale
    bkc = np.asarray(inputs["bkc"], f32)
    bvc = np.asarray(inputs["bvc"], f32)
    bd = np.asarray(inputs["bd"], f32)
    bo = np.asarray(inputs["bo"], f32)

    flags = dict(
        bq=bool(np.any(bq)), bk=bool(np.any(bk)), bv=bool(np.any(bv)),
        bqc=bool(np.any(bqc)), bkc=bool(np.any(bkc)), bd=bool(np.any(bd)),
        bo=bool(np.any(bo)),
    )

    shared = dict(
        wqT=np.ascontiguousarray(Wq.T).astype(bfnp),
        wkT=np.ascontiguousarray(Wk.T).astype(bfnp),
        wvT=np.ascontiguousarray(Wv.T).astype(bfnp),
        woT=np.ascontiguousarray(Wo.T).astype(bfnp),
        wqcT=np.ascontiguousarray(Wqc.T).astype(bfnp),
        wkcT=np.ascontiguousarray(Wkc.T).astype(bfnp),
        wvcT=np.ascontiguousarray(Wvc.T).astype(bfnp),
        wdT4=np.ascontiguousarray(np.tile(Wd.T, (4, 1))).astype(bfnp),
        bvc_t=np.concatenate(
            [np.tile(bvc, (P, 1)), np.ones((P, 1), f32)], axis=1
        ).astype(f32),
    )
    if flags["bq"]:
        shared["bq_t"] = np.ascontiguousarray(bq.reshape(H, HD).T).astype(f32)
    if flags["bk"]:
        shared["bk_t"] = np.ascontiguousarray(bk.reshape(H, HD).T).astype(f32)
    if flags["bv"]:
        shared["bv_t"] = np.ascontiguousarray(bv.reshape(H, HD).T).astype(f32)
    if flags["bqc"]:
        shared["bqc_t"] = np.tile(bqc, 4)[:, None].astype(f32)
    if flags["bkc"]:
        shared["bkc_t"] = np.tile(bkc, 4)[:, None].astype(f32)
    if flags["bd"]:
        shared["bd_t"] = bd[:, None].astype(f32)
    if flags["bo"]:
        shared["bo_t"] = np.ascontiguousarray(bo.reshape(H, HD).T).astype(f32)

    tri = (np.arange(P)[None, :] >= np.arange(P)[:, None]).astype(f32)
    ones = np.ones((P, P), f32)
    zeros = np.zeros((P, P), f32)

    in_maps = []
    for c in range(NCORES):
        m = dict(shared)
        tk0 = c * TKV
        pos_kv = np.arange(TKV) + (tk0 % L)
        m["xkvT"] = np.ascontiguousarray(x[tk0 : tk0 + TKV].T).astype(bfnp)
        m["ckv"] = np.vstack([cr[pos_kv].T, cr[pos_kv].T]).astype(f32)
        m["skv"] = np.vstack([-ci[pos_kv].T, ci[pos_kv].T]).astype(f32)

        qb = _qblocks(c)
        toks = np.concatenate(
            [np.arange(blk * P, (blk + 1) * P) + bb * L for (bb, blk) in qb]
        )
        pos_q = toks % L
        m["xqT"] = np.ascontiguousarray(x[toks].T).astype(bfnp)
        m["cq"] = np.vstack([cr[pos_q].T, cr[pos_q].T]).astype(f32)
        m["sq"] = np.vstack([-ci[pos_q].T, ci[pos_q].T]).astype(f32)

        mtab = np.zeros((NMASK * P, P), f32)
        for qi, (bb, blk) in enumerate(qb):
            for ch in range(SLOT_EXTENTS[qi]):
                blkv = ones if ch < blk else (tri if ch == blk else zeros)
                row0 = (SLOT_OFF[qi] + ch) * P
                mtab[row0 : row0 + P] = blkv
        m["mask_tab"] = mtab.astype(bfnp)
        in_maps.append(m)

    return flags, in_maps


def _assemble(results):
    out = np.empty((B, L, D), np.float32)
    for c in range(NCORES):
        oT = results[c]["outT"]
        for qi, (bb, blk) in enumerate(_qblocks(c)):
            out[bb, blk * P : (blk + 1) * P, :] = oT[:, qi * P : (qi + 1) * P].T
    return out


def run(inputs, trace=False):
    flags, in_maps = _prep_inputs(inputs)
    nc = _get_nc(flags)
    res = bass_utils.run_bass_kernel_spmd(
        nc, in_maps, core_ids=list(range(NCORES)), trace=trace
    )
    return _assemble(res.results), res


def kernel(**inputs):
    out, _ = run(inputs)
    return out


# revision 12
# speedup vs baseline: 1.0685x; 1.0415x over previous
"""Causal MLA block on 8 TRN2 NeuronCores.

Sharding (token-parallel, no all-reduce):
  * kv phase: core c computes k/v projections + RoPE + per-head compression for
    the contiguous token slice [c*512, (c+1)*512) of the flattened [4096]
    tokens, then one bf16 AllGather shares the compressed kc/vc (1 MB/core)
    with every core.
  * q/attention/out phase: core c owns 4 query blocks of 128 tokens chosen so
    causal attention work is near-identical on every core:
    batch0 blocks {c+8, c}, batch1 blocks {15-c, 7-c}  (slot order).
    It computes q projection + RoPE + compression, block-causal attention in
    the 32-dim latent space, per-head decompression, and the full output
    projection for its 512 tokens. Outputs are disjoint token rows; the host
    reassembles.

SPMD needs one static program, but each core's causal extents differ, so the
kernel processes fixed per-slot kv extents [16, 8, 16, 8] chunks and the host
bakes per-core masks (ones / causal-tri / zeros per chunk) into a mask table;
fully-masked surplus chunks contribute exactly zero to both the attention
numerator and the softmax denominator (the denominator is a ones-column
augmented onto vc, so masking the exp'd scores masks the denominator too).
For the extent-16 slots, chunks 0-7 are below the diagonal for every core and
skip the mask multiply entirely.

All matmuls run in bf16 with fp32 PSUM accumulation; RoPE and softmax in fp32.
Softmax skips max-subtraction (scores are O(1), exp cannot overflow). Weights
are pre-transposed/pre-permuted on the host so the device never transposes:
q/k projection output features are permuted within each head to [even, odd]
halves so RoPE acts on contiguous partition halves; compression weights get
the matching column permutation; the 1/sqrt(32) score scale folds into Wqc.
"""

import sys

sys.path.insert(0, "/opt/trn_rl_repo")

from contextlib import ExitStack

import numpy as np
import ml_dtypes

import concourse.bass as bass
import concourse.mybir as mybir
import concourse.tile as tile
from concourse import bacc
from concourse import bass_utils

B, L, D, H = 2, 2048, 2048, 16
HD = D // H          # 128
LAT = HD // 4        # 32
P = 128
NCORES = 8
TKV = 512            # kv tokens per core
NCH = D // P         # 16 chunks of 128
FP32 = mybir.dt.float32
BF16 = mybir.dt.bfloat16
bfnp = ml_dtypes.bfloat16
AUG = LAT + 1        # 33: vc augmented with a ones column
SLOT_EXTENTS = [16, 8, 16, 8]          # compiled kv-chunk extent per q slot
SLOT_OFF = [0, 16, 24, 40]             # mask-table column offset per slot
NMASK = 48


def _qblocks(c):
    """(batch, block_idx) per q slot; block_idx+1 <= SLOT_EXTENTS[slot]."""
    return [(0, c + 8), (0, c), (1, 15 - c), (1, 7 - c)]


def build_kernel(flags):
    nc = bacc.Bacc("TRN2", target_bir_lowering=False, debug=False, num_devices=NCORES)

    ins = {}

    def din(name, shape, dt):
        ins[name] = nc.dram_tensor(name, shape, dt, kind="ExternalInput").ap()
        return ins[name]

    xkvT = din("xkvT", [D, TKV], BF16)
    xqT = din("xqT", [D, TKV], BF16)
    wqT = din("wqT", [D, D], BF16)
    wkT = din("wkT", [D, D], BF16)
    wvT = din("wvT", [D, D], BF16)
    woT = din("woT", [D, D], BF16)
    wqcT = din("wqcT", [HD, LAT], BF16)
    wkcT = din("wkcT", [HD, LAT], BF16)
    wvcT = din("wvcT", [HD, LAT], BF16)
    wdT4 = din("wdT4", [P, HD], BF16)
    ckv = din("ckv", [P, TKV], FP32)
    skv = din("skv", [P, TKV], FP32)
    cq = din("cq", [P, TKV], FP32)
    sq = din("sq", [P, TKV], FP32)
    bvc_t = din("bvc_t", [P, AUG], FP32)       # bvc broadcast + ones column
    mask_tab = din("mask_tab", [NMASK * P, P], BF16)
    for nm in ["bq", "bk", "bv", "bo"]:
        if flags[nm]:
            din(nm + "_t", [P, H], FP32)
    for nm in ["bqc", "bkc", "bd"]:
        if flags[nm]:
            din(nm + "_t", [P, 1], FP32)

    outT = nc.dram_tensor("outT", [D, TKV], FP32, kind="ExternalOutput").ap()

    with tile.TileContext(nc) as tc, ExitStack() as ctx:
        sbc = ctx.enter_context(tc.tile_pool(name="sbc", bufs=1))
        sbx = ctx.enter_context(tc.tile_pool(name="sbx", bufs=2))
        sbw = ctx.enter_context(tc.tile_pool(name="sbw", bufs=3))
        sbr = ctx.enter_context(tc.tile_pool(name="sbr", bufs=1))
        sbt = ctx.enter_context(tc.tile_pool(name="sbt", bufs=2))
        psmm = ctx.enter_context(tc.tile_pool(name="psmm", bufs=2, space="PSUM"))
        pss = ctx.enter_context(tc.tile_pool(name="pss", bufs=4, space="PSUM"))
        psav = ctx.enter_context(tc.tile_pool(name="psav", bufs=2, space="PSUM"))
        dram = ctx.enter_context(tc.tile_pool(name="dram", bufs=1, space="DRAM"))

        kv_local = dram.tile([TKV, TKV + H * AUG], BF16)   # [512, 1040]
        kv_all = dram.tile(
            [NCORES * TKV, TKV + H * AUG], BF16, addr_space="Shared"
        )

        def w_view(w):
            return w.rearrange("(dc p) o -> p dc o", p=P)

        # ---- phase A inputs first: DMA issue order follows program order ----
        xkv_sb = sbx.tile([P, NCH, TKV], BF16, name="x_sb", tag="x_sb")
        nc.sync.dma_start(xkv_sb[:], xkvT.rearrange("(dc p) t -> p dc t", p=P))
        cs = {}
        for nm, src in [("ckv", ckv), ("skv", skv)]:
            t = sbc.tile([P, TKV], FP32, name=f"cs_{nm}", tag=f"cs_{nm}")
            nc.sync.dma_start(t[:], src)
            cs[nm] = t
        wkcT_sb = sbc.tile([HD, LAT], BF16)
        nc.sync.dma_start(wkcT_sb[:], wkcT)

        biases = {}
        for nm in ["bq", "bk", "bv", "bqc", "bkc", "bd", "bo"]:
            if flags[nm]:
                shp = [P, H] if nm in ("bq", "bk", "bv", "bo") else [P, 1]
                t = sbc.tile(shp, FP32, name=f"b_{nm}", tag=f"b_{nm}")
                nc.sync.dma_start(t[:], ins[nm + "_t"])
                biases[nm] = t

        def proj_loop(w, x_sb, wname, consume):
            """for each och: psum[o(128), t(512)] = sum_dc w.T @ x; consume it.

            Weights stream in 4-och chunks ([128, 16, 512] bf16, 2 MB) to cut
            DMA-issue count on the Sync engine.
            """
            for ochg in range(NCH // 2):
                w4 = sbw.tile([P, NCH, 2 * P], BF16, name=f"w_{wname}", tag="w4")
                nc.sync.dma_start(
                    w4[:], w_view(w)[:, :, ochg * 2 * P : (ochg + 1) * 2 * P]
                )
                for ol in range(2):
                    och = ochg * 2 + ol
                    ps = psmm.tile([P, TKV], FP32, name="ps_mm", tag="ps_mm")
                    for dc in range(NCH):
                        nc.tensor.matmul(
                            ps[:],
                            lhsT=w4[:, dc, ol * P : (ol + 1) * P],
                            rhs=x_sb[:, dc, :],
                            start=(dc == 0), stop=(dc == NCH - 1),
                        )
                    consume(och, ps)

        def rope(ps, c_t, s_t, bias, och, rot_dst):
            """rot_dst[:, och, :] (bf16) = RoPE(ps); [a;b] half layout."""
            if bias is not None:
                nc.vector.tensor_scalar_add(ps[:], ps[:], bias[:, och : och + 1])
            sw = sbt.tile([P, TKV], FP32, name="sw", tag="sw")
            nc.vector.tensor_copy(sw[0:64, :], ps[64:128, :])
            nc.vector.tensor_copy(sw[64:128, :], ps[0:64, :])
            t1 = sbt.tile([P, TKV], FP32, name="t1", tag="t1")
            t2 = sbt.tile([P, TKV], FP32, name="t2", tag="t2")
            nc.vector.tensor_mul(t1[:], ps[:], c_t[:])
            nc.vector.tensor_mul(t2[:], sw[:], s_t[:])
            nc.vector.tensor_add(rot_dst[:, och, :], t1[:], t2[:])

        def compress_cT(rot_sb, wc_sb, bias_c, dst_fn):
            """[kq]cT per head-group: 4 col-packed K=128 matmuls -> [128,512]"""
            for hg in range(4):
                ps = psmm.tile([P, TKV], FP32, name="ps_c", tag="ps_mm")
                for g in range(4):
                    h = hg * 4 + g
                    nc.tensor.matmul(
                        ps[32 * g : 32 * (g + 1), :],
                        lhsT=wc_sb[:], rhs=rot_sb[:, h, :],
                        start=True, stop=True, tile_position=(0, 32 * g),
                    )
                if bias_c is not None:
                    nc.vector.tensor_scalar_add(ps[:], ps[:], bias_c[:, 0:1])
                nc.scalar.copy(dst_fn(hg), ps[:])

        # ---- phase A: k projection + RoPE + kc compression ----
        krot = sbr.tile([P, NCH, TKV], BF16, name="rot_sb", tag="rot_sb")
        proj_loop(
            wkT, xkv_sb, "k",
            lambda och, ps: rope(ps, cs["ckv"], cs["skv"], biases.get("bk"), och, krot),
        )
        kc_sb = sbc.tile([P, 4, TKV], BF16)
        compress_cT(krot, wkcT_sb, biases.get("bkc"), lambda hg: kc_sb[:, hg, :])
        nc.gpsimd.dma_start(
            kv_local[:, 0:TKV].rearrange("(hg p) t -> p hg t", p=P), kc_sb[:]
        )

        # ---- phase A2: v projection + vc compression ----
        wvcT_sb = sbc.tile([HD, LAT], BF16)
        nc.sync.dma_start(wvcT_sb[:], wvcT)
        bvc_sb = sbc.tile([P, AUG], FP32)
        nc.sync.dma_start(bvc_sb[:], bvc_t)
        vc_sb = sbc.tile([P, TKV // P, H * AUG], BF16)  # [128, 4, 528]

        def v_consume(h, ps):
            if "bv" in biases:
                nc.vector.tensor_scalar_add(ps[:], ps[:], biases["bv"][:, h : h + 1])
            vT = sbt.tile([P, TKV], BF16, name="vT", tag="vT")
            nc.scalar.copy(vT[:], ps[:])
            for tch in range(TKV // P):
                psv = psmm.tile([P, TKV], FP32, name="ps_vc", tag="ps_mm")
                nc.tensor.matmul(
                    psv[:, 0:LAT],
                    lhsT=vT[:, tch * P : (tch + 1) * P], rhs=wvcT_sb[:],
                    start=True, stop=True,
                )
                dst = vc_sb[:, tch, h * AUG : (h + 1) * AUG]
                nc.vector.tensor_tensor(
                    dst[:, 0:LAT], psv[:, 0:LAT], bvc_sb[:, 0:LAT],
                    mybir.AluOpType.add,
                )
                nc.vector.tensor_copy(dst[:, LAT:AUG], bvc_sb[:, LAT:AUG])

        proj_loop(wvT, xkv_sb, "v", v_consume)
        nc.gpsimd.dma_start(
            kv_local[:, TKV:].rearrange("(tch p) e -> p tch e", p=P), vc_sb[:]
        )

        # ---- AllGather of compressed kv (1 MB bf16 per core) ----
        nc.gpsimd.collective_compute(
            "AllGather", mybir.AluOpType.bypass,
            replica_groups=[list(range(NCORES))],
            ins=[kv_local.opt()], outs=[kv_all.opt()],
        )

        # ---- phase B/C interleaved: per head-group, q projection (PE-dense)
        # then attention (ACT-dense) so the TensorEngine never idles long
        # enough for the HAM clock gate to re-throttle.
        xq_sb = sbx.tile([P, NCH, TKV], BF16, name="x_sb", tag="x_sb")
        nc.sync.dma_start(xq_sb[:], xqT.rearrange("(dc p) t -> p dc t", p=P))
        for nm, src in [("cq", cq), ("sq", sq)]:
            t = sbc.tile([P, TKV], FP32, name=f"cs_{nm}", tag=f"cs_{nm}")
            nc.sync.dma_start(t[:], src)
            cs[nm] = t
        wqcT_sb = sbc.tile([HD, LAT], BF16)
        nc.sync.dma_start(wqcT_sb[:], wqcT)
        mask_sb = sbc.tile([P, NMASK, P], BF16)
        nc.sync.dma_start(mask_sb[:], mask_tab.rearrange("(s p) n -> p s n", p=P))
        wdT4_sb = sbc.tile([P, HD], BF16)
        nc.sync.dma_start(wdT4_sb[:], wdT4)

        qrot = sbr.tile([P, NCH, TKV], BF16, name="rot_sb", tag="rot_sb")
        qcT_sb = [
            sbc.tile([P, TKV], BF16, name=f"qct{hg}", tag=f"qct{hg}") for hg in range(4)
        ]
        attnDT = [
            sbc.tile([P, TKV], BF16, name=f"adt{h}", tag=f"adt{h}") for h in range(H)
        ]
        kc_view = kv_all[:, 0:TKV].rearrange("(r blk p) t -> p r blk t", blk=4, p=P)

        def attention_bhg(b, hg):
            kct = sbt.tile([P, 4, TKV], BF16, name="kct", tag="kct")
            nc.gpsimd.dma_start(kct[:], kc_view[:, 4 * b : 4 * b + 4, hg, :])
            vct = sbt.tile([P, L // P, 4 * AUG], BF16, name="vct", tag="vct")
            nc.gpsimd.dma_start(
                vct[:],
                kv_all[
                    b * L : (b + 1) * L,
                    TKV + hg * 4 * AUG : TKV + (hg + 1) * 4 * AUG,
                ].rearrange("(ch p) e -> p ch e", p=P),
            )
            for qi in range(2):
                qslot = 2 * b + qi
                n_kv = SLOT_EXTENTS[qslot]
                n_grp = (n_kv + 3) // 4
                expt = [
                    sbt.tile(
                        [P, n_kv, P], BF16, name=f"expt{g}", tag=f"expt{g}",
                        bufs=2,
                    )
                    for g in range(4)
                ]
                for grp in range(n_grp):
                    nch = min(4, n_kv - grp * 4)
                    pst = [
                        pss.tile([P, TKV], FP32, name=f"ps_s{g}", tag="ps_s")
                        for g in range(4)
                    ]
                    for ci in range(nch):
                        ch = grp * 4 + ci
                        for g in range(4):
                            nc.tensor.matmul(
                                pst[g][:, ci * P : (ci + 1) * P],
                                lhsT=kct[
                                    32 * g : 32 * (g + 1),
                                    ch // 4,
                                    (ch % 4) * P : (ch % 4 + 1) * P,
                                ],
                                rhs=qcT_sb[hg][
                                    32 * g : 32 * (g + 1),
                                    qslot * P : (qslot + 1) * P,
                                ],
                                start=True, stop=True,
                                tile_position=(32 * g, 0),
                            )
                    # exp (fp32 PSUM -> bf16 SBUF); mask only where some core
                    # needs it: extent-16 slots have chunks 0-7 strictly below
                    # every core's diagonal (all-ones mask)
                    need_mask = not (n_kv == 16 and grp < 2)
                    m0 = SLOT_OFF[qslot] + grp * 4
                    msl = mask_sb[:, m0 : m0 + nch, :]
                    for g in range(4):
                        nc.scalar.activation(
                            expt[g][:, grp * 4 : grp * 4 + nch, :],
                            pst[g][:, 0 : nch * P],
                            mybir.ActivationFunctionType.Exp,
                        )
                        if need_mask:
                            nc.vector.tensor_mul(
                                expt[g][:, grp * 4 : grp * 4 + nch, :],
                                expt[g][:, grp * 4 : grp * 4 + nch, :],
                                msl,
                            )
                # av: accumulate attnT_aug over chunks; 2 heads per bank
                pav = [
                    psav.tile([P, P], FP32, name=f"ps_av{pr}", tag="ps_av")
                    for pr in range(2)
                ]
                for ch in range(n_kv):
                    for g in range(4):
                        pr, sgn = g // 2, g % 2
                        nc.tensor.matmul(
                            pav[pr][64 * sgn : 64 * sgn + AUG, :],
                            lhsT=vct[:, ch, g * AUG : (g + 1) * AUG],
                            rhs=expt[g][:, ch, :],
                            start=(ch == 0), stop=(ch == n_kv - 1),
                            tile_position=(0, 64 * sgn),
                        )
                # normalize by the ones-column sum, then decompress
                anorm = sbt.tile([P, P], BF16, name="anorm", tag="anorm")
                for g in range(4):
                    pr, sgn = g // 2, g % 2
                    den = sbt.tile([1, P], FP32, name="den", tag="den", bufs=4)
                    nc.vector.tensor_copy(
                        den[:], pav[pr][64 * sgn + LAT : 64 * sgn + LAT + 1, :]
                    )
                    rc = sbt.tile([1, P], FP32, name="rc", tag="rc", bufs=4)
                    nc.vector.reciprocal_approx_fast(rc[:], den[:])
                    rcb = sbt.tile([LAT, P], FP32, name="rcb", tag="rcb", bufs=4)
                    nc.gpsimd.partition_broadcast(rcb[:], rc[:])
                    nc.vector.tensor_mul(
                        anorm[32 * g : 32 * (g + 1), :],
                        pav[pr][64 * sgn : 64 * sgn + LAT, :],
                        rcb[:],
                    )
                for g in range(4):
                    h = hg * 4 + g
                    psd = pss.tile([P, TKV], FP32, name="ps_d", tag="ps_s")
                    nc.tensor.matmul(
                        psd[:, 0:P],
                        lhsT=wdT4_sb[32 * g : 32 * (g + 1), :],
                        rhs=anorm[32 * g : 32 * (g + 1), :],
                        start=True, stop=True, tile_position=(32 * g, 0),
                    )
                    dst = attnDT[h][:, qslot * P : (qslot + 1) * P]
                    if "bd" in biases:
                        nc.vector.tensor_scalar_add(
                            dst, psd[:, 0:P], biases["bd"][:, 0:1]
                        )
                    else:
                        nc.scalar.copy(dst, psd[:, 0:P])

        def outproj_half(bb):
            """out columns for batch bb's two q slots (256 wide)."""
            c0 = 2 * bb * P
            for ochg in range(NCH // 2):
                w4 = sbw.tile([P, NCH, 2 * P], BF16, name="w_o", tag="w4")
                nc.sync.dma_start(
                    w4[:], w_view(woT)[:, :, ochg * 2 * P : (ochg + 1) * 2 * P]
                )
                for ol in range(2):
                    och = ochg * 2 + ol
                    ps = psmm.tile([P, TKV], FP32, name="ps_o", tag="ps_mm")
                    for h in range(H):
                        nc.tensor.matmul(
                            ps[:, 0 : 2 * P],
                            lhsT=w4[:, h, ol * P : (ol + 1) * P],
                            rhs=attnDT[h][:, c0 : c0 + 2 * P],
                            start=(h == 0), stop=(h == H - 1),
                        )
                    oe = sbt.tile([P, 2 * P], FP32, name="oe", tag="oe")
                    if "bo" in biases:
                        nc.vector.tensor_scalar_add(
                            oe[:], ps[:, 0 : 2 * P], biases["bo"][:, och : och + 1]
                        )
                    else:
                        nc.scalar.copy(oe[:], ps[:, 0 : 2 * P])
                    nc.gpsimd.dma_start(
                        outT[och * P : (och + 1) * P, c0 : c0 + 2 * P], oe[:]
                    )

        for ochg in range(NCH // 2):
            w4 = sbw.tile([P, NCH, 2 * P], BF16, name="w_q", tag="w4")
            nc.sync.dma_start(
                w4[:], w_view(wqT)[:, :, ochg * 2 * P : (ochg + 1) * 2 * P]
            )
            for ol in range(2):
                och = ochg * 2 + ol
                ps = psmm.tile([P, TKV], FP32, name="ps_mm", tag="ps_mm")
                for dc in range(NCH):
                    nc.tensor.matmul(
                        ps[:],
                        lhsT=w4[:, dc, ol * P : (ol + 1) * P],
                        rhs=xq_sb[:, dc, :],
                        start=(dc == 0), stop=(dc == NCH - 1),
                    )
                rope(ps, cs["cq"], cs["sq"], biases.get("bq"), och, qrot)
        for hg in range(4):
            psq = psmm.tile([P, TKV], FP32, name="ps_c", tag="ps_mm")
            for g in range(4):
                h = hg * 4 + g
                nc.tensor.matmul(
                    psq[32 * g : 32 * (g + 1), :],
                    lhsT=wqcT_sb[:], rhs=qrot[:, h, :],
                    start=True, stop=True, tile_position=(0, 32 * g),
                )
            if "bqc" in biases:
                nc.vector.tensor_scalar_add(psq[:], psq[:], biases["bqc"][:, 0:1])
            nc.scalar.copy(qcT_sb[hg][:], psq[:])

        for b in range(B):
            for hg in range(4):
                attention_bhg(b, hg)
            # output projection for this batch's 256 columns overlaps the
            # next batch's attention input loads
            outproj_half(b)

    nc.compile()
    return nc


_NC_CACHE = {}


def _get_nc(flags):
    key = tuple(sorted(flags.items()))
    if key not in _NC_CACHE:
        _NC_CACHE[key] = build_kernel(flags)
    return _NC_CACHE[key]


def _prep_inputs(inputs):
    f32 = np.float32
    x = np.asarray(inputs["x"], f32).reshape(B * L, D)
    fc = np.asarray(inputs["freqs_cis"], f32)
    cr, ci = fc[:, :, 0], fc[:, :, 1]

    perm = np.concatenate([np.arange(0, HD, 2), np.arange(1, HD, 2)])
    permD = (np.arange(D) // HD) * HD + perm[np.arange(D) % HD]

    Wq = np.asarray(inputs["Wq"], f32)[permD]
    Wk = np.asarray(inputs["Wk"], f32)[permD]
    Wv = np.asarray(inputs["Wv"], f32)
    Wo = np.asarray(inputs["Wo"], f32)
    scale = f32(1.0 / np.sqrt(LAT))
    Wqc = np.asarray(inputs["Wqc"], f32)[:, perm] * scale
    Wkc = np.asarray(inputs["Wkc"], f32)[:, perm]
    Wvc = np.asarray(inputs["Wvc"], f32)
    Wd = np.asarray(inputs["Wd"], f32)
    bq = np.asarray(inputs["bq"], f32)[permD]
    bk = np.asarray(inputs["bk"], f32)[permD]
    bv = np.asarray(inputs["bv"], f32)
    bqc = np.asarray(inputs["bqc"], f32) * scale
    bkc = np.asarray(inputs["bkc"], f32)
    bvc = np.asarray(inputs["bvc"], f32)
    bd = np.asarray(inputs["bd"], f32)
    bo = np.asarray(inputs["bo"], f32)

    flags = dict(
        bq=bool(np.any(bq)), bk=bool(np.any(bk)), bv=bool(np.any(bv)),
        bqc=bool(np.any(bqc)), bkc=bool(np.any(bkc)), bd=bool(np.any(bd)),
        bo=bool(np.any(bo)),
    )

    shared = dict(
        wqT=np.ascontiguousarray(Wq.T).astype(bfnp),
        wkT=np.ascontiguousarray(Wk.T).astype(bfnp),
        wvT=np.ascontiguousarray(Wv.T).astype(bfnp),
        woT=np.ascontiguousarray(Wo.T).astype(bfnp),
        wqcT=np.ascontiguousarray(Wqc.T).astype(bfnp),
        wkcT=np.ascontiguousarray(Wkc.T).astype(bfnp),
        wvcT=np.ascontiguousarray(Wvc.T).astype(bfnp),
        wdT4=np.ascontiguousarray(np.tile(Wd.T, (4, 1))).astype(bfnp),
        bvc_t=np.concatenate(
            [np.tile(bvc, (P, 1)), np.ones((P, 1), f32)], axis=1
        ).astype(f32),
    )
    if flags["bq"]:
        shared["bq_t"] = np.ascontiguousarray(bq.reshape(H, HD).T).astype(f32)
    if flags["bk"]:
        shared["bk_t"] = np.ascontiguousarray(bk.reshape(H, HD).T).astype(f32)
    if flags["bv"]:
        shared["bv_t"] = np.ascontiguousarray(bv.reshape(H, HD).T).astype(f32)
    if flags["bqc"]:
        shared["bqc_t"] = np.tile(bqc, 4)[:, None].astype(f32)
    if flags["bkc"]:
        shared["bkc_t"] = np.tile(bkc, 4)[:, None].astype(f32)
    if flags["bd"]:
        shared["bd_t"] = bd[:, None].astype(f32)
    if flags["bo"]:
        shared["bo_t"] = np.ascontiguousarray(bo.reshape(H, HD).T).astype(f32)

    tri = (np.arange(P)[None, :] >= np.arange(P)[:, None]).astype(f32)
    ones = np.ones((P, P), f32)
    zeros = np.zeros((P, P), f32)

    in_maps = []
    for c in range(NCORES):
        m = dict(shared)
        tk0 = c * TKV
        pos_kv = np.arange(TKV) + (tk0 % L)
        m["xkvT"] = np.ascontiguousarray(x[tk0 : tk0 + TKV].T).astype(bfnp)
        m["ckv"] = np.vstack([cr[pos_kv].T, cr[pos_kv].T]).astype(f32)
        m["skv"] = np.vstack([-ci[pos_kv].T, ci[pos_kv].T]).astype(f32)

        qb = _qblocks(c)
        toks = np.concatenate(
            [np.arange(blk * P, (blk + 1) * P) + bb * L for (bb, blk) in qb]
        )
        pos_q = toks % L
        m["xqT"] = np.ascontiguousarray(x[toks].T).astype(bfnp)
        m["cq"] = np.vstack([cr[pos_q].T, cr[pos_q].T]).astype(f32)
        m["sq"] = np.vstack([-ci[pos_q].T, ci[pos_q].T]).astype(f32)

        mtab = np.zeros((NMASK * P, P), f32)
        for qi, (bb, blk) in enumerate(qb):
            for ch in range(SLOT_EXTENTS[qi]):
                blkv = ones if ch < blk else (tri if ch == blk else zeros)
                row0 = (SLOT_OFF[qi] + ch) * P
                mtab[row0 : row0 + P] = blkv
        m["mask_tab"] = mtab.astype(bfnp)
        in_maps.append(m)

    return flags, in_maps


def _assemble(results):
    out = np.empty((B, L, D), np.float32)
    for c in range(NCORES):
        oT = results[c]["outT"]
        for qi, (bb, blk) in enumerate(_qblocks(c)):
            out[bb, blk * P : (blk + 1) * P, :] = oT[:, qi * P : (qi + 1) * P].T
    return out


def run(inputs, trace=False):
    flags, in_maps = _prep_inputs(inputs)
    nc = _get_nc(flags)
    res = bass_utils.run_bass_kernel_spmd(
        nc, in_maps, core_ids=list(range(NCORES)), trace=trace
    )
    return _assemble(res.results), res


def kernel(**inputs):
    out, _ = run(inputs)
    return out


# revision 14
# speedup vs baseline: 1.1274x; 1.0551x over previous
"""Causal MLA block on 8 TRN2 NeuronCores.

Sharding (token-parallel, no all-reduce):
  * kv phase: core c computes k/v projections + RoPE + per-head compression for
    the contiguous token slice [c*512, (c+1)*512) of the flattened [4096]
    tokens, then one bf16 AllGather shares the compressed kc/vc (1 MB/core)
    with every core.
  * q/attention/out phase: core c owns 4 query blocks of 128 tokens chosen so
    causal attention work is near-identical on every core:
    batch0 blocks {c+8, c}, batch1 blocks {15-c, 7-c}  (slot order).
    It computes q projection + RoPE + compression, block-causal attention in
    the 32-dim latent space, per-head decompression, and the full output
    projection for its 512 tokens. Outputs are disjoint token rows; the host
    reassembles.

SPMD needs one static program, but each core's causal extents differ, so the
kernel processes fixed per-slot kv extents [16, 8, 16, 8] chunks and the host
bakes per-core masks (ones / causal-tri / zeros per chunk) into a mask table;
fully-masked surplus chunks contribute exactly zero to both the attention
numerator and the softmax denominator (the denominator is a ones-column
augmented onto vc, so masking the exp'd scores masks the denominator too).
For the extent-16 slots, chunks 0-7 are below the diagonal for every core and
skip the mask multiply entirely.

All matmuls run in bf16 with fp32 PSUM accumulation; RoPE and softmax in fp32.
Softmax skips max-subtraction (scores are O(1), exp cannot overflow). Weights
are pre-transposed/pre-permuted on the host so the device never transposes:
q/k projection output features are permuted within each head to [even, odd]
halves so RoPE acts on contiguous partition halves; compression weights get
the matching column permutation; the 1/sqrt(32) score scale folds into Wqc.
"""

import os
import sys

sys.path.insert(0, "/opt/trn_rl_repo")

from contextlib import ExitStack

import numpy as np
import ml_dtypes

import concourse.bass as bass
import concourse.mybir as mybir
import concourse.tile as tile
from concourse import bacc
from concourse import bass_utils

if os.environ.get("KOPT_LDW") == "1":
    _orig_run_command = bass_utils.run_command

    def _patched_run_command(argv, **kw):
        argv = [
            "--enable-ldw-opt=true" if a == "--enable-ldw-opt=false" else a
            for a in argv
        ]
        return _orig_run_command(argv, **kw)

    bass_utils.run_command = _patched_run_command

B, L, D, H = 2, 2048, 2048, 16
HD = D // H          # 128
LAT = HD // 4        # 32
P = 128
NCORES = 8
TKV = 512            # kv tokens per core
NCH = D // P         # 16 chunks of 128
FP32 = mybir.dt.float32
BF16 = mybir.dt.bfloat16
bfnp = ml_dtypes.bfloat16
AUG = LAT + 1        # 33: vc augmented with a ones column
SLOT_EXTENTS = [16, 8, 16, 8]          # compiled kv-chunk extent per q slot
MASKA_OFF = [0, 16]                    # per-batch: masks for short slot, ch 0-7
MASKB_OFF = [8, 24]                    # per-batch: masks for long slot, ch 8-15
NMASK = 32


def _qblocks(c):
    """(batch, block_idx) per q slot; block_idx+1 <= SLOT_EXTENTS[slot]."""
    return [(0, c + 8), (0, c), (1, 15 - c), (1, 7 - c)]


def build_kernel(flags):
    nc = bacc.Bacc("TRN2", target_bir_lowering=False, debug=False, num_devices=NCORES)

    ins = {}

    def din(name, shape, dt):
        ins[name] = nc.dram_tensor(name, shape, dt, kind="ExternalInput").ap()
        return ins[name]

    xkvT = din("xkvT", [D, TKV], BF16)
    xqT = din("xqT", [D, TKV], BF16)
    wqT = din("wqT", [D, D], BF16)
    wkT = din("wkT", [D, D], BF16)
    wvT = din("wvT", [D, D], BF16)
    woT = din("woT", [D, D], BF16)
    wqcT = din("wqcT", [HD, LAT], BF16)
    wkcT = din("wkcT", [HD, LAT], BF16)
    wvcT = din("wvcT", [HD, LAT], BF16)
    wdT4 = din("wdT4", [P, HD], BF16)
    ckv = din("ckv", [P, TKV], FP32)
    skv = din("skv", [P, TKV], FP32)
    cq = din("cq", [P, TKV], FP32)
    sq = din("sq", [P, TKV], FP32)
    bvc_t = din("bvc_t", [P, AUG], FP32)       # bvc broadcast + ones column
    mask_tab = din("mask_tab", [NMASK * P, P], BF16)
    for nm in ["bq", "bk", "bv", "bo"]:
        if flags[nm]:
            din(nm + "_t", [P, H], FP32)
    for nm in ["bqc", "bkc", "bd"]:
        if flags[nm]:
            din(nm + "_t", [P, 1], FP32)

    outT = nc.dram_tensor("outT", [TKV, D], FP32, kind="ExternalOutput").ap()

    with tile.TileContext(nc) as tc, ExitStack() as ctx:
        sbc = ctx.enter_context(tc.tile_pool(name="sbc", bufs=1))
        sbx = ctx.enter_context(tc.tile_pool(name="sbx", bufs=2))
        sbw = ctx.enter_context(tc.tile_pool(name="sbw", bufs=3))
        sbr = ctx.enter_context(tc.tile_pool(name="sbr", bufs=1))
        sbt = ctx.enter_context(tc.tile_pool(name="sbt", bufs=2))
        pss = ctx.enter_context(tc.tile_pool(name="pss", bufs=4, space="PSUM"))
        psav = ctx.enter_context(tc.tile_pool(name="psav", bufs=4, space="PSUM"))
        dram = ctx.enter_context(tc.tile_pool(name="dram", bufs=1, space="DRAM"))

        kc_local = dram.tile([TKV, TKV], BF16)
        kc_all = dram.tile([NCORES * TKV, TKV], BF16, addr_space="Shared")
        vc_local = dram.tile([TKV, H * AUG], BF16)
        vc_all = dram.tile([NCORES * TKV, H * AUG], BF16, addr_space="Shared")

        def w_view(w):
            return w.rearrange("(dc p) o -> p dc o", p=P)

        # ---- phase A inputs first: DMA issue order follows program order ----
        xkv_sb = sbx.tile([P, NCH, TKV], BF16, name="x_sb", tag="x_sb")
        nc.sync.dma_start(xkv_sb[:], xkvT.rearrange("(dc p) t -> p dc t", p=P))
        cs = {}
        for nm, src in [("ckv", ckv), ("skv", skv)]:
            t = sbc.tile([P, TKV], FP32, name=f"cs_{nm}", tag=f"cs_{nm}")
            nc.sync.dma_start(t[:], src)
            cs[nm] = t
        wkcT_sb = sbc.tile([HD, LAT], BF16)
        nc.sync.dma_start(wkcT_sb[:], wkcT)

        biases = {}
        for nm in ["bq", "bk", "bv", "bqc", "bkc", "bd"]:
            if flags[nm]:
                shp = [P, H] if nm in ("bq", "bk", "bv") else [P, 1]
                t = sbc.tile(shp, FP32, name=f"b_{nm}", tag=f"b_{nm}")
                nc.sync.dma_start(t[:], ins[nm + "_t"])
                biases[nm] = t
        if flags["bo"]:
            t = sbc.tile([P, D], FP32, name="b_bo", tag="b_bo")
            nc.sync.dma_start(t[:], ins["bo_t"])
            biases["bo"] = t

        def proj_loop(w, x_sb, wname, consume):
            """for each och: psum[o(128), t(512)] = sum_dc w.T @ x; consume."""
            for ochg in range(NCH // 2):
                w4 = sbw.tile([P, NCH, 2 * P], BF16, name=f"w_{wname}", tag="w4")
                nc.sync.dma_start(
                    w4[:], w_view(w)[:, :, ochg * 2 * P : (ochg + 1) * 2 * P]
                )
                for ol in range(2):
                    och = ochg * 2 + ol
                    ps = pss.tile([P, TKV], FP32, name="ps_mm", tag="ps_s")
                    for dc in range(NCH):
                        nc.tensor.matmul(
                            ps[:],
                            lhsT=w4[:, dc, ol * P : (ol + 1) * P],
                            rhs=x_sb[:, dc, :],
                            start=(dc == 0), stop=(dc == NCH - 1),
                        )
                    consume(och, ps)

        def rope(ps, c_t, s_t, bias, och, rot_dst):
            """rot_dst[:, och, :] (bf16) = RoPE(ps); [a;b] half layout."""
            if bias is not None:
                nc.vector.tensor_scalar_add(ps[:], ps[:], bias[:, och : och + 1])
            sw = sbt.tile([P, TKV], FP32, name="sw", tag="sw")
            nc.vector.tensor_copy(sw[0:64, :], ps[64:128, :])
            nc.vector.tensor_copy(sw[64:128, :], ps[0:64, :])
            t1 = sbt.tile([P, TKV], FP32, name="t1", tag="t1")
            t2 = sbt.tile([P, TKV], FP32, name="t2", tag="t2")
            nc.vector.tensor_mul(t1[:], ps[:], c_t[:])
            nc.vector.tensor_mul(t2[:], sw[:], s_t[:])
            nc.vector.tensor_add(rot_dst[:, och, :], t1[:], t2[:])

        # ---- phase A: k projection + RoPE + kc compression, then kc gather --
        krot = sbr.tile([P, NCH, TKV], BF16, name="rot_sb", tag="rot_sb")
        proj_loop(
            wkT, xkv_sb, "k",
            lambda och, ps: rope(ps, cs["ckv"], cs["skv"], biases.get("bk"), och, krot),
        )
        kc_sb = sbc.tile([P, 4, TKV], BF16)
        for hg in range(4):
            psk = pss.tile([P, TKV], FP32, name="ps_c", tag="ps_s")
            for g in range(4):
                nc.tensor.matmul(
                    psk[32 * g : 32 * (g + 1), :],
                    lhsT=wkcT_sb[:], rhs=krot[:, hg * 4 + g, :],
                    start=True, stop=True, tile_position=(0, 32 * g),
                )
            if "bkc" in biases:
                nc.vector.tensor_scalar_add(psk[:], psk[:], biases["bkc"][:, 0:1])
            nc.scalar.copy(kc_sb[:, hg, :], psk[:])
        nc.gpsimd.dma_start(
            kc_local[:].rearrange("(hg p) t -> p hg t", p=P), kc_sb[:]
        )
        nc.gpsimd.collective_compute(
            "AllGather", mybir.AluOpType.bypass,
            replica_groups=[list(range(NCORES))],
            ins=[kc_local.opt()], outs=[kc_all.opt()],
        )

        # ---- phase A2: v projection + vc compression (sw-pipelined) --------
        wvcT_sb = sbc.tile([HD, LAT], BF16)
        nc.sync.dma_start(wvcT_sb[:], wvcT)
        bvc_sb = sbc.tile([P, AUG], FP32)
        nc.sync.dma_start(bvc_sb[:], bvc_t)
        vc_sb = sbc.tile([P, TKV // P, H * AUG], BF16)  # [128, 4, 528]

        pending = []

        def vc_compress(h, vT):
            for tch in range(TKV // P):
                psv = pss.tile([P, TKV], FP32, name="ps_vc", tag="ps_s")
                nc.tensor.matmul(
                    psv[:, 0:LAT],
                    lhsT=vT[:, tch * P : (tch + 1) * P], rhs=wvcT_sb[:],
                    start=True, stop=True,
                )
                dst = vc_sb[:, tch, h * AUG : (h + 1) * AUG]
                nc.vector.tensor_tensor(
                    dst[:, 0:LAT], psv[:, 0:LAT], bvc_sb[:, 0:LAT],
                    mybir.AluOpType.add,
                )
                nc.vector.tensor_copy(dst[:, LAT:AUG], bvc_sb[:, LAT:AUG])

        def v_consume(h, ps):
            if "bv" in biases:
                nc.vector.tensor_scalar_add(ps[:], ps[:], biases["bv"][:, h : h + 1])
            vT = sbt.tile([P, TKV], BF16, name="vT", tag="vT", bufs=3)
            nc.scalar.copy(vT[:], ps[:])
            # defer this head's vc matmuls by one head so the eviction copy
            # never stalls the TensorEngine stream
            pending.append((h, vT))
            if len(pending) > 1:
                vc_compress(*pending.pop(0))

        proj_loop(wvT, xkv_sb, "v", v_consume)
        vc_compress(*pending.pop(0))
        nc.gpsimd.dma_start(
            vc_local[:].rearrange("(tch p) e -> p tch e", p=P), vc_sb[:]
        )
        nc.gpsimd.collective_compute(
            "AllGather", mybir.AluOpType.bypass,
            replica_groups=[list(range(NCORES))],
            ins=[vc_local.opt()], outs=[vc_all.opt()],
        )

        # ---- phase B: q projection + RoPE + qc compression (overlaps AGs) --
        xq_sb = sbx.tile([P, NCH, TKV], BF16, name="x_sb", tag="x_sb")
        nc.sync.dma_start(xq_sb[:], xqT.rearrange("(dc p) t -> p dc t", p=P))
        for nm, src in [("cq", cq), ("sq", sq)]:
            t = sbc.tile([P, TKV], FP32, name=f"cs_{nm}", tag=f"cs_{nm}")
            nc.sync.dma_start(t[:], src)
            cs[nm] = t
        wqcT_sb = sbc.tile([HD, LAT], BF16)
        nc.sync.dma_start(wqcT_sb[:], wqcT)
        mask_sb = sbc.tile([P, NMASK, P], BF16)
        nc.sync.dma_start(mask_sb[:], mask_tab.rearrange("(s p) n -> p s n", p=P))
        wdT4_sb = sbc.tile([P, HD], BF16)
        nc.sync.dma_start(wdT4_sb[:], wdT4)

        qrot = sbr.tile([P, NCH, TKV], BF16, name="rot_sb", tag="rot_sb")
        proj_loop(
            wqT, xq_sb, "q",
            lambda och, ps: rope(ps, cs["cq"], cs["sq"], biases.get("bq"), och, qrot),
        )
        qcT_sb = [
            sbc.tile([P, TKV], BF16, name=f"qct{hg}", tag=f"qct{hg}") for hg in range(4)
        ]
        for hg in range(4):
            psq = pss.tile([P, TKV], FP32, name="ps_c", tag="ps_s")
            for g in range(4):
                nc.tensor.matmul(
                    psq[32 * g : 32 * (g + 1), :],
                    lhsT=wqcT_sb[:], rhs=qrot[:, hg * 4 + g, :],
                    start=True, stop=True, tile_position=(0, 32 * g),
                )
            if "bqc" in biases:
                nc.vector.tensor_scalar_add(psq[:], psq[:], biases["bqc"][:, 0:1])
            nc.scalar.copy(qcT_sb[hg][:], psq[:])

        attnDT = [
            sbc.tile([P, TKV], BF16, name=f"adt{h}", tag=f"adt{h}") for h in range(H)
        ]
        kc_view = kc_all[:].rearrange("(r blk p) t -> p r blk t", blk=4, p=P)

        def attention_bhg(b, hg):
            """Both q slots of batch b for head-group hg in one pass.

            Per kv chunk ch (union extent 16): chunks 0-7 compute scores for
            both slots with one N=256 matmul; chunks 8-15 only the extent-16
            slot. The av matmuls share each vc stationary across both slots.
            """
            kct = sbt.tile([P, 4, TKV], BF16, name="kct", tag="kct")
            nc.gpsimd.dma_start(kct[:], kc_view[:, 4 * b : 4 * b + 4, hg, :])
            vct = sbt.tile([P, L // P, 4 * AUG], BF16, name="vct", tag="vct")
            nc.gpsimd.dma_start(
                vct[:],
                vc_all[
                    b * L : (b + 1) * L, hg * 4 * AUG : (hg + 1) * 4 * AUG
                ].rearrange("(ch p) e -> p ch e", p=P),
            )
            c2 = 2 * b * P  # qcT column offset of the slot pair
            expA = [
                sbt.tile([P, 8, 2 * P], BF16, name=f"expA{g}", tag=f"expA{g}", bufs=1)
                for g in range(4)
            ]
            expB = [
                sbt.tile([P, 8, P], BF16, name=f"expB{g}", tag=f"expB{g}", bufs=1)
                for g in range(4)
            ]
            # A-part: chunks 0-7, both slots (N=256), 2 chunks per PSUM bank
            for grp in range(4):
                pst = [
                    pss.tile([P, TKV], FP32, name=f"ps_s{g}", tag="ps_s")
                    for g in range(4)
                ]
                for ci in range(2):
                    ch = grp * 2 + ci
                    for g in range(4):
                        nc.tensor.matmul(
                            pst[g][:, ci * 2 * P : (ci + 1) * 2 * P],
                            lhsT=kct[
                                32 * g : 32 * (g + 1),
                                ch // 4, (ch % 4) * P : (ch % 4 + 1) * P,
                            ],
                            rhs=qcT_sb[hg][32 * g : 32 * (g + 1), c2 : c2 + 2 * P],
                            start=True, stop=True, tile_position=(32 * g, 0),
                        )
                for g in range(4):
                    nc.scalar.activation(
                        expA[g][:, grp * 2 : grp * 2 + 2, :],
                        pst[g][:, 0 : 4 * P],
                        mybir.ActivationFunctionType.Exp,
                    )
                    # mask the short slot's half (columns 128:256 of each chunk)
                    nc.vector.tensor_mul(
                        expA[g][:, grp * 2 : grp * 2 + 2, P : 2 * P],
                        expA[g][:, grp * 2 : grp * 2 + 2, P : 2 * P],
                        mask_sb[:, MASKA_OFF[b] + grp * 2 : MASKA_OFF[b] + grp * 2 + 2, :],
                    )
            # B-part: chunks 8-15, long slot only (N=128), 4 chunks per bank
            for grp in range(2):
                pst = [
                    pss.tile([P, TKV], FP32, name=f"ps_s{g}", tag="ps_s")
                    for g in range(4)
                ]
                for ci in range(4):
                    ch = 8 + grp * 4 + ci
                    for g in range(4):
                        nc.tensor.matmul(
                            pst[g][:, ci * P : (ci + 1) * P],
                            lhsT=kct[
                                32 * g : 32 * (g + 1),
                                ch // 4, (ch % 4) * P : (ch % 4 + 1) * P,
                            ],
                            rhs=qcT_sb[hg][32 * g : 32 * (g + 1), c2 : c2 + P],
                            start=True, stop=True, tile_position=(32 * g, 0),
                        )
                for g in range(4):
                    nc.scalar.activation(
                        expB[g][:, grp * 4 : grp * 4 + 4, :],
                        pst[g][:, 0 : 4 * P],
                        mybir.ActivationFunctionType.Exp,
                    )
                    nc.vector.tensor_mul(
                        expB[g][:, grp * 4 : grp * 4 + 4, :],
                        expB[g][:, grp * 4 : grp * 4 + 4, :],
                        mask_sb[:, MASKB_OFF[b] + grp * 4 : MASKB_OFF[b] + grp * 4 + 4, :],
                    )
            # av: accumulate attnT_aug; one vc stationary serves both slots
            pav = [
                [
                    psav.tile([P, P], FP32, name=f"ps_av{sl}{pr}", tag="ps_av")
                    for pr in range(2)
                ]
                for sl in range(2)
            ]
            for ch in range(16):
                for g in range(4):
                    pr, sgn = g // 2, g % 2
                    lhs = vct[:, ch, g * AUG : (g + 1) * AUG]
                    if ch < 8:
                        rhs0 = expA[g][:, ch, 0:P]
                        rhs1 = expA[g][:, ch, P : 2 * P]
                        nc.tensor.matmul(
                            pav[0][pr][64 * sgn : 64 * sgn + AUG, :],
                            lhsT=lhs, rhs=rhs0,
                            start=(ch == 0), stop=(ch == 15),
                            tile_position=(0, 64 * sgn),
                        )
                        nc.tensor.matmul(
                            pav[1][pr][64 * sgn : 64 * sgn + AUG, :],
                            lhsT=lhs, rhs=rhs1,
                            start=(ch == 0), stop=(ch == 7),
                            tile_position=(0, 64 * sgn),
                        )
                    else:
                        nc.tensor.matmul(
                            pav[0][pr][64 * sgn : 64 * sgn + AUG, :],
                            lhsT=lhs, rhs=expB[g][:, ch - 8, :],
                            start=False, stop=(ch == 15),
                            tile_position=(0, 64 * sgn),
                        )
            # normalize by the ones-column sum, then decompress; per slot
            for sl in range(2):
                qslot = 2 * b + sl
                anorm = sbt.tile([P, P], BF16, name="anorm", tag="anorm")
                for g in range(4):
                    pr, sgn = g // 2, g % 2
                    den = sbt.tile([1, P], FP32, name="den", tag="den", bufs=4)
                    nc.vector.tensor_copy(
                        den[:], pav[sl][pr][64 * sgn + LAT : 64 * sgn + LAT + 1, :]
                    )
                    rc = sbt.tile([1, P], FP32, name="rc", tag="rc", bufs=4)
                    nc.vector.reciprocal_approx_fast(rc[:], den[:])
                    rcb = sbt.tile([LAT, P], FP32, name="rcb", tag="rcb", bufs=4)
                    nc.gpsimd.partition_broadcast(rcb[:], rc[:])
                    nc.vector.tensor_mul(
                        anorm[32 * g : 32 * (g + 1), :],
                        pav[sl][pr][64 * sgn : 64 * sgn + LAT, :],
                        rcb[:],
                    )
                for g in range(4):
                    h = hg * 4 + g
                    psd = pss.tile([P, TKV], FP32, name="ps_d", tag="ps_s")
                    nc.tensor.matmul(
                        psd[:, 0:P],
                        lhsT=wdT4_sb[32 * g : 32 * (g + 1), :],
                        rhs=anorm[32 * g : 32 * (g + 1), :],
                        start=True, stop=True, tile_position=(32 * g, 0),
                    )
                    dst = attnDT[h][:, qslot * P : (qslot + 1) * P]
                    if "bd" in biases:
                        nc.vector.tensor_scalar_add(
                            dst, psd[:, 0:P], biases["bd"][:, 0:1]
                        )
                    else:
                        nc.scalar.copy(dst, psd[:, 0:P])

        def outproj_half(bb):
            """out rows for batch bb's two q slots (t-blocks 2bb, 2bb+1).

            [t, o] layout: stationary = attnDT block (K=128, FWL-eligible),
            reused across all four 512-wide o slices -> LDWEIGHTS amortized.
            """
            acc = [
                [
                    (pss if osl < 2 else psav).tile(
                        [P, TKV], FP32, name=f"ps_u{tl}{osl}",
                        tag=("ps_s" if osl < 2 else "ps_av"),
                    )
                    for osl in range(4)
                ]
                for tl in range(2)
            ]
            for h in range(H):
                woh = sbw.tile([P, D], BF16, name="woh", tag="woh")
                nc.sync.dma_start(
                    woh[:], woT[h * P : (h + 1) * P, :].rearrange("(x p) o -> p x o", p=P)[:, 0, :]
                )
                for tl in range(2):
                    tblk = 2 * bb + tl
                    for osl in range(4):
                        nc.tensor.matmul(
                            acc[tl][osl][:],
                            lhsT=attnDT[h][:, tblk * P : (tblk + 1) * P],
                            rhs=woh[:, osl * TKV : (osl + 1) * TKV],
                            start=(h == 0), stop=(h == H - 1),
                        )
            for tl in range(2):
                tblk = 2 * bb + tl
                for osl in range(4):
                    oe = sbt.tile([P, TKV], FP32, name="oe", tag="oe")
                    if "bo" in biases:
                        nc.vector.tensor_tensor(
                            oe[:], acc[tl][osl][:],
                            biases["bo"][:, osl * TKV : (osl + 1) * TKV],
                            mybir.AluOpType.add,
                        )
                    else:
                        nc.scalar.copy(oe[:], acc[tl][osl][:])
                    nc.gpsimd.dma_start(
                        outT[tblk * P : (tblk + 1) * P, osl * TKV : (osl + 1) * TKV],
                        oe[:],
                    )

        for b in range(B):
            for hg in range(4):
                attention_bhg(b, hg)
            outproj_half(b)

    nc.compile()
    return nc


_NC_CACHE = {}


def _get_nc(flags):
    key = tuple(sorted(flags.items()))
    if key not in _NC_CACHE:
        _NC_CACHE[key] = build_kernel(flags)
    return _NC_CACHE[key]


def _prep_inputs(inputs):
    f32 = np.float32
    x = np.asarray(inputs["x"], f32).reshape(B * L, D)
    fc = np.asarray(inputs["freqs_cis"], f32)
    cr, ci = fc[:, :, 0], fc[:, :, 1]

    perm = np.concatenate([np.arange(0, HD, 2), np.arange(1, HD, 2)])
    permD = (np.arange(D) // HD) * HD + perm[np.arange(D) % HD]

    Wq = np.asarray(inputs["Wq"], f32)[permD]
    Wk = np.asarray(inputs["Wk"], f32)[permD]
    Wv = np.asarray(inputs["Wv"], f32)
    Wo = np.asarray(inputs["Wo"], f32)
    scale = f32(1.0 / np.sqrt(LAT))
    Wqc = np.asarray(inputs["Wqc"], f32)[:, perm] * scale
    Wkc = np.asarray(inputs["Wkc"], f32)[:, perm]
    Wvc = np.asarray(inputs["Wvc"], f32)
    Wd = np.asarray(inputs["Wd"], f32)
    bq = np.asarray(inputs["bq"], f32)[permD]
    bk = np.asarray(inputs["bk"], f32)[permD]
    bv = np.asarray(inputs["bv"], f32)
    bqc = np.asarray(inputs["bqc"], f32) * scale
    bkc = np.asarray(inputs["bkc"], f32)
    bvc = np.asarray(inputs["bvc"], f32)
    bd = np.asarray(inputs["bd"], f32)
    bo = np.asarray(inputs["bo"], f32)

    flags = dict(
        bq=bool(np.any(bq)), bk=bool(np.any(bk)), bv=bool(np.any(bv)),
        bqc=bool(np.any(bqc)), bkc=bool(np.any(bkc)), bd=bool(np.any(bd)),
        bo=bool(np.any(bo)),
    )

    shared = dict(
        wqT=np.ascontiguousarray(Wq.T).astype(bfnp),
        wkT=np.ascontiguousarray(Wk.T).astype(bfnp),
        wvT=np.ascontiguousarray(Wv.T).astype(bfnp),
        woT=np.ascontiguousarray(Wo.T).astype(bfnp),
        wqcT=np.ascontiguousarray(Wqc.T).astype(bfnp),
        wkcT=np.ascontiguousarray(Wkc.T).astype(bfnp),
        wvcT=np.ascontiguousarray(Wvc.T).astype(bfnp),
        wdT4=np.ascontiguousarray(np.tile(Wd.T, (4, 1))).astype(bfnp),
        bvc_t=np.concatenate(
            [np.tile(bvc, (P, 1)), np.ones((P, 1), f32)], axis=1
        ).astype(f32),
    )
    if flags["bq"]:
        shared["bq_t"] = np.ascontiguousarray(bq.reshape(H, HD).T).astype(f32)
    if flags["bk"]:
        shared["bk_t"] = np.ascontiguousarray(bk.reshape(H, HD).T).astype(f32)
    if flags["bv"]:
        shared["bv_t"] = np.ascontiguousarray(bv.reshape(H, HD).T).astype(f32)
    if flags["bqc"]:
        shared["bqc_t"] = np.tile(bqc, 4)[:, None].astype(f32)
    if flags["bkc"]:
        shared["bkc_t"] = np.tile(bkc, 4)[:, None].astype(f32)
    if flags["bd"]:
        shared["bd_t"] = bd[:, None].astype(f32)
    if flags["bo"]:
        shared["bo_t"] = np.tile(bo[None, :], (P, 1)).astype(f32)

    tri = (np.arange(P)[None, :] >= np.arange(P)[:, None]).astype(f32)
    ones = np.ones((P, P), f32)
    zeros = np.zeros((P, P), f32)

    in_maps = []
    for c in range(NCORES):
        m = dict(shared)
        tk0 = c * TKV
        pos_kv = np.arange(TKV) + (tk0 % L)
        m["xkvT"] = np.ascontiguousarray(x[tk0 : tk0 + TKV].T).astype(bfnp)
        m["ckv"] = np.vstack([cr[pos_kv].T, cr[pos_kv].T]).astype(f32)
        m["skv"] = np.vstack([-ci[pos_kv].T, ci[pos_kv].T]).astype(f32)

        qb = _qblocks(c)
        toks = np.concatenate(
            [np.arange(blk * P, (blk + 1) * P) + bb * L for (bb, blk) in qb]
        )
        pos_q = toks % L
        m["xqT"] = np.ascontiguousarray(x[toks].T).astype(bfnp)
        m["cq"] = np.vstack([cr[pos_q].T, cr[pos_q].T]).astype(f32)
        m["sq"] = np.vstack([-ci[pos_q].T, ci[pos_q].T]).astype(f32)

        mtab = np.zeros((NMASK * P, P), f32)
        for bb in range(B):
            blk_long = qb[2 * bb][1]
            blk_short = qb[2 * bb + 1][1]
            for ch in range(8):
                blkv = ones if ch < blk_short else (
                    tri if ch == blk_short else zeros
                )
                row0 = (MASKA_OFF[bb] + ch) * P
                mtab[row0 : row0 + P] = blkv
            for ch in range(8, 16):
                blkv = ones if ch < blk_long else (
                    tri if ch == blk_long else zeros
                )
                row0 = (MASKB_OFF[bb] + ch - 8) * P
                mtab[row0 : row0 + P] = blkv
        m["mask_tab"] = mtab.astype(bfnp)
        in_maps.append(m)

    return flags, in_maps


def _assemble(results):
    out = np.empty((B, L, D), np.float32)
    for c in range(NCORES):
        oU = results[c]["outT"]
        for qi, (bb, blk) in enumerate(_qblocks(c)):
            out[bb, blk * P : (blk + 1) * P, :] = oU[qi * P : (qi + 1) * P, :]
    return out


def run(inputs, trace=False):
    flags, in_maps = _prep_inputs(inputs)
    nc = _get_nc(flags)
    res = bass_utils.run_bass_kernel_spmd(
        nc, in_maps, core_ids=list(range(NCORES)), trace=trace
    )
    return _assemble(res.results), res


def kernel(**inputs):
    out, _ = run(inputs)
    return out
